# revision 10
# baseline (speedup 1.0000x reference)
"""Trainium2 Bass kernel for nn_ConfounderStackLayers.

Computation (per batch row b, confounder c):
    h0 = relu(x @ W0[c].T + b0[c])        # [B, H0]
    h1 = relu(h0 @ W1[c].T + b1[c])       # [B, H1]
    out[b, c] = h1 @ W2[c, 0] + b2[c]     # scalar head

Sharding: data-parallel over batch across 8 cores (2048 rows each), weights
replicated, no collectives.

v2 design (from trace analysis of the f32r/alternating-evict baseline):
  - bf16 operands: enables FWL (fast weight load) so LDWEIGHTS hides under
    the 512-cycle matmul streaming; f32r paid a serial ~170ns LDWEIGHTS.
  - Single evict engine (ACT): every PSUM->SBUF evict (relu+bias) runs on
    the scalar engine.  All PE instruction deps then collapse onto the ACT
    sem and all ACT deps onto the PE sem, so the walrus one-wait-per-
    instruction budget is met with almost no "touch" scaffolding.
  - Layer 2 col-tiling: the per-(c,kt) scalar-head matmuls have M=1 but
    cost a full 512-row stream each.  They are batched per batch-chunk and
    issued as 4 concurrent column-tiles (tile_position via out base
    partition 32j), overlapping 4 chains of 4 accumulation matmuls ->
    ~4 slots instead of 16.  Group c=2j+cc lands on PSUM partition 32j+cc.
  - PE warmup: dummy matmuls run while the first DMA chunks land so the
    real stream starts at the warm (fast) PE clock.
  - DMA order: bias, then x/weights in consumption order, w2 last.
"""

import os
from contextlib import ExitStack

import numpy as np

import concourse.bass as bass
import concourse.mybir as mybir
import concourse.tile as tile
from concourse.tile_rust import add_dep_helper
from concourse.bass_utils import run_bass_kernel_spmd

NCORES = 8
B, C, D, H0, H1 = 16384, 8, 256, 512, 256
BS = B // NCORES          # 2048 batch rows per core
BC = 512                  # batch chunk (one psum bank of fp32)
NB = BS // BC             # 4
KT0, MT0 = D // 128, H0 // 128    # 2, 4
KT1, MT1 = H0 // 128, H1 // 128   # 4, 2
KT2 = H1 // 128                   # 2

WCOLS = KT0 * H0 + KT1 * H1       # per-c combined weight columns (w0 then w1)
W0C = KT0 * H0
W2_COLS = 4 * 2 * KT2 * 2         # [j, cc, kt, m] -> 32
B0_OFF, B1_OFF, B2_OFF = 0, C * MT0, C * MT0 + C * MT1
BIAS_COLS = B2_OFF + 1            # 49
NWARM = 8                         # PE warm-up dummy matmuls

MM_MODE = os.environ.get("KERNEL_MM_MODE", "bf16")

_CACHE = {}


def _build(mm: str) -> bass.Bass:
    f32 = mybir.dt.float32
    if mm == "bf16":
        wdt = mybir.dt.bfloat16
    elif mm == "f32r":
        wdt = mybir.dt.float32r
    else:
        wdt = f32
    relu = mybir.ActivationFunctionType.Relu
    copy_f = mybir.ActivationFunctionType.Copy

    nc = bass.Bass(trn_type="TRN2")
    xd = nc.dram_tensor("xd", [128, NB, KT0, BC], wdt, kind="ExternalInput")
    w2d = nc.dram_tensor("w2d", [128, W2_COLS], wdt, kind="ExternalInput")
    wcat = nc.dram_tensor("wcat", [C, 128, WCOLS], wdt, kind="ExternalInput")
    biasd = nc.dram_tensor("biasd", [128, BIAS_COLS], f32, kind="ExternalInput")
    outT3 = nc.dram_tensor("outT3", [4, 2, BS], f32, kind="ExternalOutput")

    with tile.TileContext(nc) as tc, ExitStack() as ctx:
        consts = ctx.enter_context(tc.tile_pool(name="consts", bufs=1))
        h0p = ctx.enter_context(tc.tile_pool(name="h0", bufs=2))
        h1p = ctx.enter_context(tc.tile_pool(name="h1", bufs=8))
        ps0p = ctx.enter_context(tc.tile_pool(name="ps0", bufs=3, space="PSUM"))
        ps1p = ctx.enter_context(tc.tile_pool(name="ps1", bufs=2, space="PSUM"))
        ps2p = ctx.enter_context(tc.tile_pool(name="ps2", bufs=2, space="PSUM"))
        pewp = ctx.enter_context(tc.tile_pool(name="pew", bufs=1, space="PSUM"))

        bias = consts.tile([128, BIAS_COLS], f32, tag="bias")
        xs = consts.tile([128, NB, KT0, BC], wdt, tag="xs")
        wts = [
            consts.tile([128, WCOLS], wdt, name=f"w_{c}", tag=f"w_{c}")
            for c in range(C)
        ]
        w2s = consts.tile([128, W2_COLS], wdt, tag="w2s")
        zeros = consts.tile([1, BC], wdt, tag="zeros")
        outt = consts.tile([128, NB, BC], f32, tag="outt")
        act_scr = consts.tile([1, 64], f32, tag="act_scr")
        gp_scr = consts.tile([1, 8], f32, tag="gp_scr")
        pewarm = pewp.tile([128, BC], f32, tag="pewarm")

        mset = nc.gpsimd.memset(zeros[:, :], 0.0)

        # Input DMAs in consumption order on the HWDGE ring.
        funnel_deps = []
        funnel_deps.append(nc.sync.dma_start(out=bias, in_=biasd[:, :]))
        dma_order = [("x", 0), ("w0", 0), ("w1", 0), ("w0", 1), ("w1", 1),
                     ("x", 1), ("w0", 2), ("w1", 2), ("w0", 3), ("w1", 3),
                     ("x", 2), ("w0", 4), ("w1", 4), ("w0", 5), ("w1", 5),
                     ("x", 3), ("w0", 6), ("w1", 6), ("w0", 7), ("w1", 7)]
        for kind, i in dma_order:
            if kind == "x":
                funnel_deps.append(nc.sync.dma_start(out=xs[:, i], in_=xd[:, i]))
            elif kind == "w0":
                funnel_deps.append(
                    nc.sync.dma_start(out=wts[i][:, 0:W0C], in_=wcat[i][:, 0:W0C])
                )
            else:
                funnel_deps.append(
                    nc.sync.dma_start(
                        out=wts[i][:, W0C:WCOLS], in_=wcat[i][:, W0C:WCOLS]
                    )
                )
        funnel_deps.append(nc.sync.dma_start(out=w2s, in_=w2d[:, :]))

        def xs_ap(kt, bi):
            return xs[:, bi, kt, :]

        def w0_ap(c, kt, mt):
            lo = kt * H0 + mt * 128
            return wts[c][:, lo:lo + 128]

        def w1_ap(c, kt, mt):
            lo = W0C + kt * H1 + mt * 128
            return wts[c][:, lo:lo + 128]

        def b0_ap(c, mt):
            return bias[:, B0_OFF + c * MT0 + mt:B0_OFF + c * MT0 + mt + 1]

        def b1_ap(c, mt):
            return bias[:, B1_OFF + c * MT1 + mt:B1_OFF + c * MT1 + mt + 1]

        b2_ap = bias[:, B2_OFF:B2_OFF + 1]

        def w2_ap(c, kt):
            j, cc = c // 2, c % 2
            off = (j * 4 + cc * 2 + kt) * 2
            return w2s[:, off:off + 2]

        state = {"pe_prev": None, "pe_cnt": 0, "act_cnt": 0, "first_evict": True}

        def act_touch(src_ap):
            j = state["act_cnt"]
            state["act_cnt"] += 1
            return nc.scalar.activation(act_scr[0:1, j:j + 1], src_ap, copy_f)

        def pe_pin(inst):
            # no-sync scheduler edge chaining PE program order
            if state["pe_prev"] is not None:
                add_dep_helper(inst.ins, state["pe_prev"].ins, False, "pe order")
            state["pe_prev"] = inst
            return inst

        def pe_touch(col_ap):
            j = state["pe_cnt"]
            state["pe_cnt"] += 1
            if col_ap.dtype == mybir.dt.float32r:
                col_ap = col_ap.bitcast(mybir.dt.float32)
            t = nc.tensor.matmul(
                pewarm[0:1, j:j + 1], lhsT=col_ap, rhs=col_ap,
                start=True, stop=True,
            )
            return pe_pin(t)

        z_lhs = zeros[0:1, 0:128]
        z_rhs = zeros[0:1, 0:BC]
        if wdt == mybir.dt.float32r:
            z_lhs = z_lhs.bitcast(f32)
            z_rhs = z_rhs.bitcast(f32)

        # PE warm-up while the first DMAs land.
        for _ in range(NWARM):
            pe_pin(nc.tensor.matmul(
                pewarm[:, :], lhsT=z_lhs, rhs=z_rhs, start=True, stop=True))

        act_bias_touch = act_touch(bias[0:1, 0:1])

        def evict(dst_ap, src_ps, bias_ap, with_relu=True, touch_first=False):
            # A slot-reusing evict would carry two waits (own-sem slot release
            # + PE psum); the touch absorbs the PE wait first.
            t = None
            if touch_first:
                t = act_touch(src_ps[0:1, 0:1])
            if with_relu:
                e = nc.scalar.activation(dst_ap, src_ps, relu, bias=bias_ap)
            else:
                e = nc.scalar.add(dst_ap, src_ps, bias_ap)
            if t is not None:
                add_dep_helper(e.ins, t.ins, False, "touch before evict")
            if state["first_evict"]:
                state["first_evict"] = False
                add_dep_helper(
                    (t or e).ins, act_bias_touch.ins, False, "bias touch first")
            return e

        last_act = None
        for bi in range(NB):
            ps2 = ps2p.tile([128, BC], f32)
            if bi < 2:
                # zero the full bank once so the out-evict never reads
                # uninitialized PSUM in the unused partition rows
                pe_pin(nc.tensor.matmul(
                    ps2[:, :], lhsT=z_lhs, rhs=z_rhs, start=True, stop=True))
            xt = pe_touch(xs[:, bi, 0, 0:1])
            h1s = []
            for c in range(C):
                if bi == 0:
                    wt = pe_touch(wts[c][:, 0:1])
                h0 = h0p.tile([128, KT1, BC], wdt)
                for mt in range(MT0):
                    ps0 = ps0p.tile([128, BC], f32)
                    for kt in range(KT0):
                        mmi = pe_pin(nc.tensor.matmul(
                            ps0,
                            lhsT=w0_ap(c, kt, mt),
                            rhs=xs_ap(kt, bi),
                            start=(kt == 0),
                            stop=(kt == KT0 - 1),
                        ))
                        if bi == 0 and mt == 0 and kt == 0:
                            add_dep_helper(mmi.ins, wt.ins, False, "wt first")
                        if c == 0 and mt == 0 and kt == 0:
                            add_dep_helper(mmi.ins, xt.ins, False, "xt first")
                    evict(h0[:, mt, :], ps0, b0_ap(c, mt), touch_first=(mt == 0))
                if bi == 0:
                    wt1 = pe_touch(wts[c][:, W0C:W0C + 1])
                h1 = h1p.tile([128, KT2, BC], wdt)
                h1s.append(h1)
                for mt in range(MT1):
                    ps1 = ps1p.tile([128, BC], f32)
                    for kt in range(KT1):
                        mm1 = pe_pin(nc.tensor.matmul(
                            ps1,
                            lhsT=w1_ap(c, kt, mt),
                            rhs=h0[:, kt, :],
                            start=(kt == 0),
                            stop=(kt == KT1 - 1),
                        ))
                        if bi == 0 and mt == 0 and kt == 0:
                            add_dep_helper(mm1.ins, wt1.ins, False, "w1t first")
                    evict(h1[:, mt, :], ps1, b1_ap(c, mt))
            # Layer 2: batched col-tiled scalar heads.  Column group
            # j = c//2 accumulates c in {2j, 2j+1} into psum partitions
            # [32j : 32j+2); the 4 groups' chains overlap in the PE array.
            if bi == 0:
                w2t = pe_touch(w2s[:, 0:1])
            for c in range(C):
                j, cc = c // 2, c % 2
                for kt in range(KT2):
                    mm2 = pe_pin(nc.tensor.matmul(
                        ps2[32 * j:32 * j + 2, :],
                        lhsT=w2_ap(c, kt),
                        rhs=h1s[c][:, kt, :],
                        start=(cc == 0 and kt == 0),
                        stop=(cc == 1 and kt == KT2 - 1),
                        tile_position=(0, 32 * j),
                    ))
                    if bi == 0 and c == 0 and kt == 0:
                        add_dep_helper(mm2.ins, w2t.ins, False, "w2t first")
            last_act = evict(outt[:, bi, :], ps2, b2_ap, with_relu=False)
            # gpsimd touch absorbs the ACT (out-evict) wait so the DMA
            # triggers carry only their SW-queue sem.
            gpt = nc.gpsimd.tensor_copy(
                gp_scr[0:1, bi:bi + 1], outt[0:1, bi, 0:1])
            for j in range(4):
                od = nc.gpsimd.dma_start(
                    out=outT3[j][:, bi * BC:(bi + 1) * BC],
                    in_=outt[32 * j:32 * j + 2, bi, :],
                )
                add_dep_helper(od.ins, gpt.ins, False, "gp touch before dma")
                funnel_deps.append(od)

        funnel_deps += [mset, last_act, state["pe_prev"]]
        for dep in funnel_deps:
            n = nc.sync.nop()
            add_dep_helper(n.ins, dep.ins, True, "drain funnel")
    return nc


def _np_wdt(mm: str):
    if mm == "bf16":
        import ml_dtypes

        return ml_dtypes.bfloat16
    return np.float32


def kernel(x, W0, b0, W1, b1, W2, b2, trace=False):
    mm = MM_MODE
    key = ("nc", mm)
    if key not in _CACHE:
        _CACHE[key] = _build(mm)
    nc = _CACHE[key]
    wnp = _np_wdt(mm)

    x = np.ascontiguousarray(np.asarray(x, dtype=np.float32))
    W0 = np.asarray(W0, dtype=np.float32)
    W1 = np.asarray(W1, dtype=np.float32)
    W2 = np.asarray(W2, dtype=np.float32)
    b0 = np.asarray(b0, dtype=np.float32)
    b1 = np.asarray(b1, dtype=np.float32)
    b2 = np.asarray(b2, dtype=np.float32)

    # Combined per-c weight block: [C, 128, KT0*H0 + KT1*H1] where
    # wcat[c, p, kt*H0 + h] = W0[c, h, kt*128+p] and
    # wcat[c, p, KT0*H0 + kt*H1 + o] = W1[c, o, kt*128+p].
    wcat = np.empty((C, 128, WCOLS), dtype=np.float32)
    w0v = wcat[:, :, :W0C].reshape(C, 128, KT0, H0)
    w0v[...] = W0.reshape(C, H0, KT0, 128).transpose(0, 3, 2, 1)
    w1v = wcat[:, :, W0C:].reshape(C, 128, KT1, H1)
    w1v[...] = W1.reshape(C, H1, KT1, 128).transpose(0, 3, 2, 1)
    wcat = np.ascontiguousarray(wcat).astype(wnp)

    # Layer-2 col-tiled lhsT tiles: for (c, kt) a [128, 2] tile at column
    # (j*4 + cc*2 + kt)*2, with the W2 slice in column m=cc, 0 in the other.
    w2part = np.zeros((128, W2_COLS), dtype=np.float32)
    for c in range(C):
        j, cc = c // 2, c % 2
        for kt in range(KT2):
            col = (j * 4 + cc * 2 + kt) * 2 + cc
            w2part[:, col] = W2[c, 0, kt * 128:(kt + 1) * 128]
    w2part = w2part.astype(wnp)

    biasd = np.zeros((128, BIAS_COLS), dtype=np.float32)
    biasd[:, B0_OFF:B0_OFF + C * MT0] = (
        b0.reshape(C, MT0, 128).transpose(2, 0, 1).reshape(128, C * MT0)
    )
    biasd[:, B1_OFF:B1_OFF + C * MT1] = (
        b1.reshape(C, MT1, 128).transpose(2, 0, 1).reshape(128, C * MT1)
    )
    for c in range(C):
        biasd[32 * (c // 2) + c % 2, B2_OFF] = b2[c]

    xTfull = np.ascontiguousarray(x.T)  # [D, B] fp32
    in_maps = []
    for s in range(NCORES):
        xsh = xTfull[:, s * BS:(s + 1) * BS]          # [D, BS]
        # xd[p, bi, kt, b] = x[s*BS + bi*BC + b, kt*128 + p]
        xdn = np.ascontiguousarray(
            xsh.reshape(KT0, 128, NB, BC).transpose(1, 2, 0, 3)
        ).astype(wnp)
        in_maps.append(
            {"xd": xdn, "w2d": w2part, "wcat": wcat, "biasd": biasd}
        )

    res = run_bass_kernel_spmd(
        nc, in_maps, core_ids=list(range(NCORES)), trace=trace
    )
    _CACHE["last_result"] = res

    out = np.empty((B, C), dtype=np.float32)
    for s in range(NCORES):
        o3 = res.results[s]["outT3"]  # [4, 2, BS]
        for c in range(C):
            out[s * BS:(s + 1) * BS, c] = o3[c // 2, c % 2]
    return out


# revision 19
# speedup vs baseline: 1.0131x; 1.0131x over previous
"""Trainium2 Bass kernel for nn_ConfounderStackLayers.

Computation (per batch row b, confounder c):
    h0 = relu(x @ W0[c].T + b0[c])        # [B, H0]
    h1 = relu(h0 @ W1[c].T + b1[c])       # [B, H1]
    out[b, c] = h1 @ W2[c, 0] + b2[c]     # scalar head

Sharding: data-parallel over batch across 8 cores (2048 rows each), weights
replicated, no collectives.

v2 design (from trace analysis of the f32r/alternating-evict baseline):
  - bf16 operands: enables FWL (fast weight load) so LDWEIGHTS hides under
    the 512-cycle matmul streaming; f32r paid a serial ~170ns LDWEIGHTS.
  - Single evict engine (ACT): every PSUM->SBUF evict (relu+bias) runs on
    the scalar engine.  All PE instruction deps then collapse onto the ACT
    sem and all ACT deps onto the PE sem, so the walrus one-wait-per-
    instruction budget is met with almost no "touch" scaffolding.
  - Layer 2 col-tiling: the per-(c,kt) scalar-head matmuls have M=1 but
    cost a full 512-row stream each.  They are batched per batch-chunk and
    issued as 4 concurrent column-tiles (tile_position via out base
    partition 32j), overlapping 4 chains of 4 accumulation matmuls ->
    ~4 slots instead of 16.  Group c=2j+cc lands on PSUM partition 32j+cc.
  - PE warmup: dummy matmuls run while the first DMA chunks land so the
    real stream starts at the warm (fast) PE clock.
  - DMA order: bias, then x/weights in consumption order, w2 last.
"""

import os
from contextlib import ExitStack

import numpy as np

import concourse.bass as bass
import concourse.mybir as mybir
import concourse.tile as tile
from concourse.tile_rust import add_dep_helper
from concourse.bass_utils import run_bass_kernel_spmd

NCORES = 8
B, C, D, H0, H1 = 16384, 8, 256, 512, 256
BS = B // NCORES          # 2048 batch rows per core
BC = 512                  # batch chunk (one psum bank of fp32)
NB = BS // BC             # 4
KT0, MT0 = D // 128, H0 // 128    # 2, 4
KT1, MT1 = H0 // 128, H1 // 128   # 4, 2
KT2 = H1 // 128                   # 2

WCOLS = KT0 * H0 + KT1 * H1       # per-c combined weight columns (w0 then w1)
W0C = KT0 * H0
W2_COLS = 4 * 2 * KT2 * 2         # [j, cc, kt, m] -> 32
B0_OFF, B1_OFF, B2_OFF = 0, C * MT0, C * MT0 + C * MT1
BIAS_COLS = B2_OFF + 1            # 49
NWARM = 8                         # PE warm-up dummy matmuls

MM_MODE = os.environ.get("KERNEL_MM_MODE", "bf16")

_CACHE = {}


def _build(mm: str) -> bass.Bass:
    f32 = mybir.dt.float32
    if mm == "bf16":
        wdt = mybir.dt.bfloat16
    elif mm == "f32r":
        wdt = mybir.dt.float32r
    else:
        wdt = f32
    relu = mybir.ActivationFunctionType.Relu
    copy_f = mybir.ActivationFunctionType.Copy

    nc = bass.Bass(trn_type="TRN2")
    xd = nc.dram_tensor("xd", [128, NB, KT0, BC], wdt, kind="ExternalInput")
    w2d = nc.dram_tensor("w2d", [128, W2_COLS], wdt, kind="ExternalInput")
    wcat = nc.dram_tensor("wcat", [C, 128, WCOLS], wdt, kind="ExternalInput")
    biasd = nc.dram_tensor("biasd", [128, BIAS_COLS], f32, kind="ExternalInput")
    outT3 = nc.dram_tensor("outT3", [4, 2, BS], f32, kind="ExternalOutput")

    with tile.TileContext(nc) as tc, ExitStack() as ctx:
        consts = ctx.enter_context(tc.tile_pool(name="consts", bufs=1))
        h0p = ctx.enter_context(tc.tile_pool(name="h0", bufs=2))
        h1p = ctx.enter_context(tc.tile_pool(name="h1", bufs=8))
        ps0p = ctx.enter_context(tc.tile_pool(name="ps0", bufs=3, space="PSUM"))
        ps1p = ctx.enter_context(tc.tile_pool(name="ps1", bufs=2, space="PSUM"))
        ps2p = ctx.enter_context(tc.tile_pool(name="ps2", bufs=2, space="PSUM"))
        pewp = ctx.enter_context(tc.tile_pool(name="pew", bufs=1, space="PSUM"))

        bias = consts.tile([128, BIAS_COLS], f32, tag="bias")
        xs = consts.tile([128, NB, KT0, BC], wdt, tag="xs")
        wts = [
            consts.tile([128, WCOLS], wdt, name=f"w_{c}", tag=f"w_{c}")
            for c in range(C)
        ]
        w2s = consts.tile([128, W2_COLS], wdt, tag="w2s")
        zeros = consts.tile([1, BC], wdt, tag="zeros")
        outt = consts.tile([128, NB, BC], f32, tag="outt")
        act_scr = consts.tile([1, 64], f32, tag="act_scr")
        dve_scr = consts.tile([1, 64], f32, tag="dve_scr")
        gp_scr = consts.tile([1, 8], f32, tag="gp_scr")
        pewarm = pewp.tile([128, BC], f32, tag="pewarm")

        mset = nc.gpsimd.memset(zeros[:, :], 0.0)

        # Input DMAs in consumption order on the HWDGE ring.
        funnel_deps = []
        funnel_deps.append(nc.sync.dma_start(out=bias, in_=biasd[:, :]))
        dma_order = [("x", 0), ("w0", 0), ("w1", 0), ("w0", 1), ("w1", 1),
                     ("x", 1), ("w0", 2), ("w1", 2), ("w0", 3), ("w1", 3),
                     ("x", 2), ("w0", 4), ("w1", 4), ("w0", 5), ("w1", 5),
                     ("x", 3), ("w0", 6), ("w1", 6), ("w0", 7), ("w1", 7)]
        for kind, i in dma_order:
            if kind == "x":
                funnel_deps.append(nc.sync.dma_start(out=xs[:, i], in_=xd[:, i]))
            elif kind == "w0":
                funnel_deps.append(
                    nc.sync.dma_start(out=wts[i][:, 0:W0C], in_=wcat[i][:, 0:W0C])
                )
            else:
                funnel_deps.append(
                    nc.sync.dma_start(
                        out=wts[i][:, W0C:WCOLS], in_=wcat[i][:, W0C:WCOLS]
                    )
                )
        funnel_deps.append(nc.sync.dma_start(out=w2s, in_=w2d[:, :]))

        def xs_ap(kt, bi):
            return xs[:, bi, kt, :]

        def w0_ap(c, kt, mt):
            lo = kt * H0 + mt * 128
            return wts[c][:, lo:lo + 128]

        def w1_ap(c, kt, mt):
            lo = W0C + kt * H1 + mt * 128
            return wts[c][:, lo:lo + 128]

        def b0_ap(c, mt):
            return bias[:, B0_OFF + c * MT0 + mt:B0_OFF + c * MT0 + mt + 1]

        def b1_ap(c, mt):
            return bias[:, B1_OFF + c * MT1 + mt:B1_OFF + c * MT1 + mt + 1]

        b2_ap = bias[:, B2_OFF:B2_OFF + 1]

        def w2_ap(c, kt):
            j, cc = c // 2, c % 2
            off = (j * 4 + cc * 2 + kt) * 2
            return w2s[:, off:off + 2]

        state = {"pe_prev": None, "pe_cnt": 0, "act_cnt": 0, "dve_cnt": 0,
                 "first_evict": {"act": True, "dve": True}}
        add_op = mybir.AluOpType.add
        max_op = mybir.AluOpType.max

        def act_touch(src_ap):
            j = state["act_cnt"]
            state["act_cnt"] += 1
            return nc.scalar.activation(act_scr[0:1, j:j + 1], src_ap, copy_f)

        def dve_touch(src_ap):
            j = state["dve_cnt"]
            state["dve_cnt"] += 1
            return nc.vector.tensor_copy(dve_scr[0:1, j:j + 1], src_ap)

        def pe_pin(inst):
            # no-sync scheduler edge chaining PE program order
            if state["pe_prev"] is not None:
                add_dep_helper(inst.ins, state["pe_prev"].ins, False, "pe order")
            state["pe_prev"] = inst
            return inst

        def pe_touch(col_ap):
            j = state["pe_cnt"]
            state["pe_cnt"] += 1
            if col_ap.dtype == mybir.dt.float32r:
                col_ap = col_ap.bitcast(mybir.dt.float32)
            t = nc.tensor.matmul(
                pewarm[0:1, j:j + 1], lhsT=col_ap, rhs=col_ap,
                start=True, stop=True,
            )
            return pe_pin(t)

        z_lhs = zeros[0:1, 0:128]
        z_rhs = zeros[0:1, 0:BC]
        if wdt == mybir.dt.float32r:
            z_lhs = z_lhs.bitcast(f32)
            z_rhs = z_rhs.bitcast(f32)

        # PE warm-up while the first DMAs land.
        for _ in range(NWARM):
            pe_pin(nc.tensor.matmul(
                pewarm[:, :], lhsT=z_lhs, rhs=z_rhs, start=True, stop=True))

        act_bias_touch = act_touch(bias[0:1, 0:1])
        dve_bias_touch = dve_touch(bias[0:1, 0:1])

        def evict(engine, dst_ap, src_ps, bias_ap, with_relu=True,
                  touch_first=False):
            # A slot-reusing evict would carry two waits (own-sem slot release
            # + PE psum); the touch absorbs the PE wait first.
            t = None
            if touch_first:
                t = act_touch(src_ps[0:1, 0:1]) if engine == "act" \
                    else dve_touch(src_ps[0:1, 0:1])
            if engine == "act":
                if with_relu:
                    e = nc.scalar.activation(dst_ap, src_ps, relu, bias=bias_ap)
                else:
                    e = nc.scalar.add(dst_ap, src_ps, bias_ap)
            else:
                e = nc.vector.tensor_scalar(
                    dst_ap, src_ps, bias_ap, 0.0, add_op, max_op)
            if engine == "dve":
                state["last_dve"] = e
            if t is not None:
                add_dep_helper(e.ins, t.ins, False, "touch before evict")
            if state["first_evict"][engine]:
                state["first_evict"][engine] = False
                bt = act_bias_touch if engine == "act" else dve_bias_touch
                add_dep_helper((t or e).ins, bt.ins, False, "bias touch first")
            return e

        last_act = None
        for bi in range(NB):
            ps2 = ps2p.tile([128, BC], f32)
            if bi < 2:
                # zero the full bank once so the out-evict never reads
                # uninitialized PSUM in the unused partition rows
                pe_pin(nc.tensor.matmul(
                    ps2[:, :], lhsT=z_lhs, rhs=z_rhs, start=True, stop=True))
            xt = pe_touch(xs[:, bi, 0, 0:1])
            h1s = []
            for c in range(C):
                if bi == 0:
                    wt = pe_touch(wts[c][:, 0:1])
                h0 = h0p.tile([128, KT1, BC], wdt)
                for mt in range(MT0):
                    ps0 = ps0p.tile([128, BC], f32)
                    for kt in range(KT0):
                        mmi = pe_pin(nc.tensor.matmul(
                            ps0,
                            lhsT=w0_ap(c, kt, mt),
                            rhs=xs_ap(kt, bi),
                            start=(kt == 0),
                            stop=(kt == KT0 - 1),
                        ))
                        if bi == 0 and mt == 0 and kt == 0:
                            add_dep_helper(mmi.ins, wt.ins, False, "wt first")
                        if c == 0 and mt == 0 and kt == 0:
                            add_dep_helper(mmi.ins, xt.ins, False, "xt first")
                    evict("act", h0[:, mt, :], ps0, b0_ap(c, mt),
                          touch_first=(mt == 0))
                if bi == 0:
                    wt1 = pe_touch(wts[c][:, W0C:W0C + 1])
                # Absorb the ACT (h0 write) wait on PE so the first L1 matmul
                # carries only its ps1 slot-release (DVE) wait.
                h0t = pe_touch(h0[0:1, 0, 0:1])
                h1 = h1p.tile([128, KT2, BC], wdt)
                h1s.append(h1)
                for mt in range(MT1):
                    ps1 = ps1p.tile([128, BC], f32)
                    for kt in range(KT1):
                        mm1 = pe_pin(nc.tensor.matmul(
                            ps1,
                            lhsT=w1_ap(c, kt, mt),
                            rhs=h0[:, kt, :],
                            start=(kt == 0),
                            stop=(kt == KT1 - 1),
                        ))
                        if bi == 0 and mt == 0 and kt == 0:
                            add_dep_helper(mm1.ins, wt1.ins, False, "w1t first")
                        if mt == 0 and kt == 0:
                            add_dep_helper(mm1.ins, h0t.ins, False, "h0t first")
                    evict("dve", h1[:, mt, :], ps1, b1_ap(c, mt),
                          touch_first=(mt == 0))
            # Layer 2: batched col-tiled scalar heads.  Column group
            # j = c//2 accumulates c in {2j, 2j+1} into psum partitions
            # [32j : 32j+2); the 4 groups' chains overlap in the PE array.
            if bi == 0:
                w2t = pe_touch(w2s[:, 0:1])
            if bi >= 2:
                # Absorb the ps2 slot-release (ACT out-evict of bi-2) so the
                # first L2 matmul carries only its h1 (DVE) wait.
                pe_touch(outt[0:1, bi - 2, 0:1])
            for c in range(C):
                j, cc = c // 2, c % 2
                for kt in range(KT2):
                    mm2 = pe_pin(nc.tensor.matmul(
                        ps2[32 * j:32 * j + 2, :],
                        lhsT=w2_ap(c, kt),
                        rhs=h1s[c][:, kt, :],
                        start=(cc == 0 and kt == 0),
                        stop=(cc == 1 and kt == KT2 - 1),
                        tile_position=(0, 32 * j),
                    ))
                    if bi == 0 and c == 0 and kt == 0:
                        add_dep_helper(mm2.ins, w2t.ins, False, "w2t first")
            last_act = evict("act", outt[:, bi, :], ps2, b2_ap, with_relu=False)
            # gpsimd touch absorbs the ACT (out-evict) wait so the DMA
            # triggers carry only their SW-queue sem.
            gpt = nc.gpsimd.tensor_copy(
                gp_scr[0:1, bi:bi + 1], outt[0:1, bi, 0:1])
            for j in range(4):
                od = nc.gpsimd.dma_start(
                    out=outT3[j][:, bi * BC:(bi + 1) * BC],
                    in_=outt[32 * j:32 * j + 2, bi, :],
                )
                add_dep_helper(od.ins, gpt.ins, False, "gp touch before dma")
                funnel_deps.append(od)

        funnel_deps += [mset, last_act, state["last_dve"], state["pe_prev"]]
        for dep in funnel_deps:
            n = nc.sync.nop()
            add_dep_helper(n.ins, dep.ins, True, "drain funnel")
    return nc


def _np_wdt(mm: str):
    if mm == "bf16":
        import ml_dtypes

        return ml_dtypes.bfloat16
    return np.float32


def kernel(x, W0, b0, W1, b1, W2, b2, trace=False):
    mm = MM_MODE
    key = ("nc", mm)
    if key not in _CACHE:
        _CACHE[key] = _build(mm)
    nc = _CACHE[key]
    wnp = _np_wdt(mm)

    x = np.ascontiguousarray(np.asarray(x, dtype=np.float32))
    W0 = np.asarray(W0, dtype=np.float32)
    W1 = np.asarray(W1, dtype=np.float32)
    W2 = np.asarray(W2, dtype=np.float32)
    b0 = np.asarray(b0, dtype=np.float32)
    b1 = np.asarray(b1, dtype=np.float32)
    b2 = np.asarray(b2, dtype=np.float32)

    # Combined per-c weight block: [C, 128, KT0*H0 + KT1*H1] where
    # wcat[c, p, kt*H0 + h] = W0[c, h, kt*128+p] and
    # wcat[c, p, KT0*H0 + kt*H1 + o] = W1[c, o, kt*128+p].
    wcat = np.empty((C, 128, WCOLS), dtype=np.float32)
    w0v = wcat[:, :, :W0C].reshape(C, 128, KT0, H0)
    w0v[...] = W0.reshape(C, H0, KT0, 128).transpose(0, 3, 2, 1)
    w1v = wcat[:, :, W0C:].reshape(C, 128, KT1, H1)
    w1v[...] = W1.reshape(C, H1, KT1, 128).transpose(0, 3, 2, 1)
    wcat = np.ascontiguousarray(wcat).astype(wnp)

    # Layer-2 col-tiled lhsT tiles: for (c, kt) a [128, 2] tile at column
    # (j*4 + cc*2 + kt)*2, with the W2 slice in column m=cc, 0 in the other.
    w2part = np.zeros((128, W2_COLS), dtype=np.float32)
    for c in range(C):
        j, cc = c // 2, c % 2
        for kt in range(KT2):
            col = (j * 4 + cc * 2 + kt) * 2 + cc
            w2part[:, col] = W2[c, 0, kt * 128:(kt + 1) * 128]
    w2part = w2part.astype(wnp)

    biasd = np.zeros((128, BIAS_COLS), dtype=np.float32)
    biasd[:, B0_OFF:B0_OFF + C * MT0] = (
        b0.reshape(C, MT0, 128).transpose(2, 0, 1).reshape(128, C * MT0)
    )
    biasd[:, B1_OFF:B1_OFF + C * MT1] = (
        b1.reshape(C, MT1, 128).transpose(2, 0, 1).reshape(128, C * MT1)
    )
    for c in range(C):
        biasd[32 * (c // 2) + c % 2, B2_OFF] = b2[c]

    xTfull = np.ascontiguousarray(x.T)  # [D, B] fp32
    in_maps = []
    for s in range(NCORES):
        xsh = xTfull[:, s * BS:(s + 1) * BS]          # [D, BS]
        # xd[p, bi, kt, b] = x[s*BS + bi*BC + b, kt*128 + p]
        xdn = np.ascontiguousarray(
            xsh.reshape(KT0, 128, NB, BC).transpose(1, 2, 0, 3)
        ).astype(wnp)
        in_maps.append(
            {"xd": xdn, "w2d": w2part, "wcat": wcat, "biasd": biasd}
        )

    res = run_bass_kernel_spmd(
        nc, in_maps, core_ids=list(range(NCORES)), trace=trace
    )
    _CACHE["last_result"] = res

    out = np.empty((B, C), dtype=np.float32)
    for s in range(NCORES):
        o3 = res.results[s]["outT3"]  # [4, 2, BS]
        for c in range(C):
            out[s * BS:(s + 1) * BS, c] = o3[c // 2, c % 2]
    return out


# revision 20
# speedup vs baseline: 1.0246x; 1.0114x over previous
"""Trainium2 Bass kernel for nn_ConfounderStackLayers.

Computation (per batch row b, confounder c):
    h0 = relu(x @ W0[c].T + b0[c])        # [B, H0]
    h1 = relu(h0 @ W1[c].T + b1[c])       # [B, H1]
    out[b, c] = h1 @ W2[c, 0] + b2[c]     # scalar head

Sharding: data-parallel over batch across 8 cores (2048 rows each), weights
replicated, no collectives.

v2 design (from trace analysis of the f32r/alternating-evict baseline):
  - bf16 operands: enables FWL (fast weight load) so LDWEIGHTS hides under
    the 512-cycle matmul streaming; f32r paid a serial ~170ns LDWEIGHTS.
  - Single evict engine (ACT): every PSUM->SBUF evict (relu+bias) runs on
    the scalar engine.  All PE instruction deps then collapse onto the ACT
    sem and all ACT deps onto the PE sem, so the walrus one-wait-per-
    instruction budget is met with almost no "touch" scaffolding.
  - Layer 2 col-tiling: the per-(c,kt) scalar-head matmuls have M=1 but
    cost a full 512-row stream each.  They are batched per batch-chunk and
    issued as 4 concurrent column-tiles (tile_position via out base
    partition 32j), overlapping 4 chains of 4 accumulation matmuls ->
    ~4 slots instead of 16.  Group c=2j+cc lands on PSUM partition 32j+cc.
  - PE warmup: dummy matmuls run while the first DMA chunks land so the
    real stream starts at the warm (fast) PE clock.
  - DMA order: bias, then x/weights in consumption order, w2 last.
"""

import os
from contextlib import ExitStack

import numpy as np

import concourse.bass as bass
import concourse.mybir as mybir
import concourse.tile as tile
from concourse.tile_rust import add_dep_helper
from concourse.bass_utils import run_bass_kernel_spmd

NCORES = 8
B, C, D, H0, H1 = 16384, 8, 256, 512, 256
BS = B // NCORES          # 2048 batch rows per core
BC = 512                  # batch chunk (one psum bank of fp32)
NB = BS // BC             # 4
KT0, MT0 = D // 128, H0 // 128    # 2, 4
KT1, MT1 = H0 // 128, H1 // 128   # 4, 2
KT2 = H1 // 128                   # 2

WCOLS = KT0 * H0 + KT1 * H1       # per-c combined weight columns (w0 then w1)
W0C = KT0 * H0
W2_COLS = 4 * 2 * KT2 * 2         # [j, cc, kt, m] -> 32
B0_OFF, B1_OFF, B2_OFF = 0, C * MT0, C * MT0 + C * MT1
BIAS_COLS = B2_OFF + 1            # 49
NWARM = 8                         # PE warm-up dummy matmuls

MM_MODE = os.environ.get("KERNEL_MM_MODE", "bf16")

_CACHE = {}


def _build(mm: str) -> bass.Bass:
    f32 = mybir.dt.float32
    if mm == "bf16":
        wdt = mybir.dt.bfloat16
    elif mm == "f32r":
        wdt = mybir.dt.float32r
    else:
        wdt = f32
    relu = mybir.ActivationFunctionType.Relu
    copy_f = mybir.ActivationFunctionType.Copy

    nc = bass.Bass(trn_type="TRN2")
    xd = nc.dram_tensor("xd", [128, NB, KT0, BC], wdt, kind="ExternalInput")
    w2d = nc.dram_tensor("w2d", [128, W2_COLS], wdt, kind="ExternalInput")
    wcat = nc.dram_tensor("wcat", [C, 128, WCOLS], wdt, kind="ExternalInput")
    biasd = nc.dram_tensor("biasd", [128, BIAS_COLS], f32, kind="ExternalInput")
    outT2 = nc.dram_tensor("outT2", [2, 4, BS], f32, kind="ExternalOutput")

    with tile.TileContext(nc) as tc, ExitStack() as ctx:
        consts = ctx.enter_context(tc.tile_pool(name="consts", bufs=1))
        h0p = ctx.enter_context(tc.tile_pool(name="h0", bufs=2))
        h1p = ctx.enter_context(tc.tile_pool(name="h1", bufs=8))
        ps0p = ctx.enter_context(tc.tile_pool(name="ps0", bufs=4, space="PSUM"))
        ps1p = ctx.enter_context(tc.tile_pool(name="ps1", bufs=2, space="PSUM"))
        ps2p = ctx.enter_context(tc.tile_pool(name="ps2", bufs=1, space="PSUM"))
        pewp = ctx.enter_context(tc.tile_pool(name="pew", bufs=1, space="PSUM"))

        bias = consts.tile([128, BIAS_COLS], f32, tag="bias")
        xs = consts.tile([128, NB, KT0, BC], wdt, tag="xs")
        wts = [
            consts.tile([128, WCOLS], wdt, name=f"w_{c}", tag=f"w_{c}")
            for c in range(C)
        ]
        w2s = consts.tile([128, W2_COLS], wdt, tag="w2s")
        zeros = consts.tile([1, BC], wdt, tag="zeros")
        outt = consts.tile([128, NB, BC], f32, tag="outt")
        act_scr = consts.tile([1, 64], f32, tag="act_scr")
        dve_scr = consts.tile([1, 64], f32, tag="dve_scr")
        gp_scr = consts.tile([1, 8], f32, tag="gp_scr")
        pewarm = pewp.tile([128, BC], f32, tag="pewarm")

        mset = nc.gpsimd.memset(zeros[:, :], 0.0)

        # Input DMAs in consumption order on the HWDGE ring.
        funnel_deps = []
        funnel_deps.append(nc.sync.dma_start(out=bias, in_=biasd[:, :]))
        dma_order = [("x", 0), ("w0", 0), ("w1", 0), ("w0", 1), ("w1", 1),
                     ("x", 1), ("w0", 2), ("w1", 2), ("w0", 3), ("w1", 3),
                     ("x", 2), ("w0", 4), ("w1", 4), ("w0", 5), ("w1", 5),
                     ("x", 3), ("w0", 6), ("w1", 6), ("w0", 7), ("w1", 7)]
        for kind, i in dma_order:
            if kind == "x":
                funnel_deps.append(nc.sync.dma_start(out=xs[:, i], in_=xd[:, i]))
            elif kind == "w0":
                funnel_deps.append(
                    nc.sync.dma_start(out=wts[i][:, 0:W0C], in_=wcat[i][:, 0:W0C])
                )
            else:
                funnel_deps.append(
                    nc.sync.dma_start(
                        out=wts[i][:, W0C:WCOLS], in_=wcat[i][:, W0C:WCOLS]
                    )
                )
        funnel_deps.append(nc.sync.dma_start(out=w2s, in_=w2d[:, :]))

        def xs_ap(kt, bi):
            return xs[:, bi, kt, :]

        def w0_ap(c, kt, mt):
            lo = kt * H0 + mt * 128
            return wts[c][:, lo:lo + 128]

        def w1_ap(c, kt, mt):
            lo = W0C + kt * H1 + mt * 128
            return wts[c][:, lo:lo + 128]

        def b0_ap(c, mt):
            return bias[:, B0_OFF + c * MT0 + mt:B0_OFF + c * MT0 + mt + 1]

        def b1_ap(c, mt):
            return bias[:, B1_OFF + c * MT1 + mt:B1_OFF + c * MT1 + mt + 1]

        b2_ap = bias[:, B2_OFF:B2_OFF + 1]

        def w2_ap(c, kt):
            j, cc = c // 2, c % 2
            off = (j * 4 + cc * 2 + kt) * 2
            return w2s[:, off:off + 2]

        state = {"pe_prev": None, "pe_cnt": 0, "act_cnt": 0, "dve_cnt": 0,
                 "first_evict": {"act": True, "dve": True}}
        add_op = mybir.AluOpType.add
        max_op = mybir.AluOpType.max

        def act_touch(src_ap):
            j = state["act_cnt"]
            state["act_cnt"] += 1
            return nc.scalar.activation(act_scr[0:1, j:j + 1], src_ap, copy_f)

        def dve_touch(src_ap):
            j = state["dve_cnt"]
            state["dve_cnt"] += 1
            return nc.vector.tensor_copy(dve_scr[0:1, j:j + 1], src_ap)

        def pe_pin(inst):
            # no-sync scheduler edge chaining PE program order
            if state["pe_prev"] is not None:
                add_dep_helper(inst.ins, state["pe_prev"].ins, False, "pe order")
            state["pe_prev"] = inst
            return inst

        def pe_touch(col_ap):
            j = state["pe_cnt"]
            state["pe_cnt"] += 1
            if col_ap.dtype == mybir.dt.float32r:
                col_ap = col_ap.bitcast(mybir.dt.float32)
            t = nc.tensor.matmul(
                pewarm[0:1, j:j + 1], lhsT=col_ap, rhs=col_ap,
                start=True, stop=True,
            )
            return pe_pin(t)

        z_lhs = zeros[0:1, 0:128]
        z_rhs = zeros[0:1, 0:BC]
        if wdt == mybir.dt.float32r:
            z_lhs = z_lhs.bitcast(f32)
            z_rhs = z_rhs.bitcast(f32)

        # PE warm-up while the first DMAs land.
        for _ in range(NWARM):
            pe_pin(nc.tensor.matmul(
                pewarm[:, :], lhsT=z_lhs, rhs=z_rhs, start=True, stop=True))

        act_bias_touch = act_touch(bias[0:1, 0:1])
        dve_bias_touch = dve_touch(bias[0:1, 0:1])

        def evict(engine, dst_ap, src_ps, bias_ap, with_relu=True,
                  touch_first=False):
            # A slot-reusing evict would carry two waits (own-sem slot release
            # + PE psum); the touch absorbs the PE wait first.
            t = None
            if touch_first:
                t = act_touch(src_ps[0:1, 0:1]) if engine == "act" \
                    else dve_touch(src_ps[0:1, 0:1])
            if engine == "act":
                if with_relu:
                    e = nc.scalar.activation(dst_ap, src_ps, relu, bias=bias_ap)
                else:
                    e = nc.scalar.add(dst_ap, src_ps, bias_ap)
            else:
                e = nc.vector.tensor_scalar(
                    dst_ap, src_ps, bias_ap, 0.0, add_op, max_op)
            if engine == "dve":
                state["last_dve"] = e
            if t is not None:
                add_dep_helper(e.ins, t.ins, False, "touch before evict")
            if state["first_evict"][engine]:
                state["first_evict"][engine] = False
                bt = act_bias_touch if engine == "act" else dve_bias_touch
                add_dep_helper((t or e).ins, bt.ins, False, "bias touch first")
            return e

        last_act = None
        for bi in range(NB):
            ps2 = ps2p.tile([128, BC], f32)
            if bi == 0:
                # zero the full bank once so the out-evict never reads
                # uninitialized PSUM in the unused partition rows
                pe_pin(nc.tensor.matmul(
                    ps2[:, :], lhsT=z_lhs, rhs=z_rhs, start=True, stop=True))
            xt = pe_touch(xs[:, bi, 0, 0:1])
            h1s = []
            for c in range(C):
                if bi == 0:
                    wt = pe_touch(wts[c][:, 0:1])
                h0 = h0p.tile([128, KT1, BC], wdt)
                for mt in range(MT0):
                    ps0 = ps0p.tile([128, BC], f32)
                    for kt in range(KT0):
                        mmi = pe_pin(nc.tensor.matmul(
                            ps0,
                            lhsT=w0_ap(c, kt, mt),
                            rhs=xs_ap(kt, bi),
                            start=(kt == 0),
                            stop=(kt == KT0 - 1),
                        ))
                        if bi == 0 and mt == 0 and kt == 0:
                            add_dep_helper(mmi.ins, wt.ins, False, "wt first")
                        if c == 0 and mt == 0 and kt == 0:
                            add_dep_helper(mmi.ins, xt.ins, False, "xt first")
                    evict("act", h0[:, mt, :], ps0, b0_ap(c, mt),
                          touch_first=(mt == 0))
                if bi == 0:
                    wt1 = pe_touch(wts[c][:, W0C:W0C + 1])
                # Absorb the ACT (h0 write) wait on PE so the first L1 matmul
                # carries only its ps1 slot-release (DVE) wait.
                h0t = pe_touch(h0[0:1, 0, 0:1])
                h1 = h1p.tile([128, KT2, BC], wdt)
                h1s.append(h1)
                for mt in range(MT1):
                    ps1 = ps1p.tile([128, BC], f32)
                    for kt in range(KT1):
                        mm1 = pe_pin(nc.tensor.matmul(
                            ps1,
                            lhsT=w1_ap(c, kt, mt),
                            rhs=h0[:, kt, :],
                            start=(kt == 0),
                            stop=(kt == KT1 - 1),
                        ))
                        if bi == 0 and mt == 0 and kt == 0:
                            add_dep_helper(mm1.ins, wt1.ins, False, "w1t first")
                        if mt == 0 and kt == 0:
                            add_dep_helper(mm1.ins, h0t.ins, False, "h0t first")
                    evict("dve", h1[:, mt, :], ps1, b1_ap(c, mt),
                          touch_first=(mt == 0))
            # Layer 2: batched col-tiled scalar heads.  Column group
            # j = c//2 accumulates c in {2j, 2j+1} into psum partitions
            # [32j : 32j+2); the 4 groups' chains overlap in the PE array.
            if bi == 0:
                w2t = pe_touch(w2s[:, 0:1])
            if bi >= 1:
                # Absorb the ps2 slot-release (ACT out-evict of bi-1) so the
                # first L2 matmul carries only its h1 (DVE) wait.
                pe_touch(outt[0:1, bi - 1, 0:1])
            for c in range(C):
                j, cc = c // 2, c % 2
                for kt in range(KT2):
                    mm2 = pe_pin(nc.tensor.matmul(
                        ps2[32 * j:32 * j + 2, :],
                        lhsT=w2_ap(c, kt),
                        rhs=h1s[c][:, kt, :],
                        start=(cc == 0 and kt == 0),
                        stop=(cc == 1 and kt == KT2 - 1),
                        tile_position=(0, 32 * j),
                    ))
                    if bi == 0 and c == 0 and kt == 0:
                        add_dep_helper(mm2.ins, w2t.ins, False, "w2t first")
            last_act = evict("act", outt[:, bi, :], ps2, b2_ap, with_relu=False)
            # gpsimd touch absorbs the ACT (out-evict) wait so the DMA
            # triggers carry only their SW-queue sem.
            gpt = nc.gpsimd.tensor_copy(
                gp_scr[0:1, bi:bi + 1], outt[0:1, bi, 0:1])
            for m in range(2):
                od = nc.gpsimd.dma_start(
                    out=outT2[m][:, bi * BC:(bi + 1) * BC],
                    in_=outt[m:97 + m:32, bi, :],
                )
                add_dep_helper(od.ins, gpt.ins, False, "gp touch before dma")
                funnel_deps.append(od)

        funnel_deps += [mset, last_act, state["last_dve"], state["pe_prev"]]
        for dep in funnel_deps:
            n = nc.sync.nop()
            add_dep_helper(n.ins, dep.ins, True, "drain funnel")
    return nc


def _np_wdt(mm: str):
    if mm == "bf16":
        import ml_dtypes

        return ml_dtypes.bfloat16
    return np.float32


def kernel(x, W0, b0, W1, b1, W2, b2, trace=False):
    mm = MM_MODE
    key = ("nc", mm)
    if key not in _CACHE:
        _CACHE[key] = _build(mm)
    nc = _CACHE[key]
    wnp = _np_wdt(mm)

    x = np.ascontiguousarray(np.asarray(x, dtype=np.float32))
    W0 = np.asarray(W0, dtype=np.float32)
    W1 = np.asarray(W1, dtype=np.float32)
    W2 = np.asarray(W2, dtype=np.float32)
    b0 = np.asarray(b0, dtype=np.float32)
    b1 = np.asarray(b1, dtype=np.float32)
    b2 = np.asarray(b2, dtype=np.float32)

    # Combined per-c weight block: [C, 128, KT0*H0 + KT1*H1] where
    # wcat[c, p, kt*H0 + h] = W0[c, h, kt*128+p] and
    # wcat[c, p, KT0*H0 + kt*H1 + o] = W1[c, o, kt*128+p].
    wcat = np.empty((C, 128, WCOLS), dtype=np.float32)
    w0v = wcat[:, :, :W0C].reshape(C, 128, KT0, H0)
    w0v[...] = W0.reshape(C, H0, KT0, 128).transpose(0, 3, 2, 1)
    w1v = wcat[:, :, W0C:].reshape(C, 128, KT1, H1)
    w1v[...] = W1.reshape(C, H1, KT1, 128).transpose(0, 3, 2, 1)
    wcat = np.ascontiguousarray(wcat).astype(wnp)

    # Layer-2 col-tiled lhsT tiles: for (c, kt) a [128, 2] tile at column
    # (j*4 + cc*2 + kt)*2, with the W2 slice in column m=cc, 0 in the other.
    w2part = np.zeros((128, W2_COLS), dtype=np.float32)
    for c in range(C):
        j, cc = c // 2, c % 2
        for kt in range(KT2):
            col = (j * 4 + cc * 2 + kt) * 2 + cc
            w2part[:, col] = W2[c, 0, kt * 128:(kt + 1) * 128]
    w2part = w2part.astype(wnp)

    biasd = np.zeros((128, BIAS_COLS), dtype=np.float32)
    biasd[:, B0_OFF:B0_OFF + C * MT0] = (
        b0.reshape(C, MT0, 128).transpose(2, 0, 1).reshape(128, C * MT0)
    )
    biasd[:, B1_OFF:B1_OFF + C * MT1] = (
        b1.reshape(C, MT1, 128).transpose(2, 0, 1).reshape(128, C * MT1)
    )
    for c in range(C):
        biasd[32 * (c // 2) + c % 2, B2_OFF] = b2[c]

    xTfull = np.ascontiguousarray(x.T)  # [D, B] fp32
    in_maps = []
    for s in range(NCORES):
        xsh = xTfull[:, s * BS:(s + 1) * BS]          # [D, BS]
        # xd[p, bi, kt, b] = x[s*BS + bi*BC + b, kt*128 + p]
        xdn = np.ascontiguousarray(
            xsh.reshape(KT0, 128, NB, BC).transpose(1, 2, 0, 3)
        ).astype(wnp)
        in_maps.append(
            {"xd": xdn, "w2d": w2part, "wcat": wcat, "biasd": biasd}
        )

    res = run_bass_kernel_spmd(
        nc, in_maps, core_ids=list(range(NCORES)), trace=trace
    )
    _CACHE["last_result"] = res

    out = np.empty((B, C), dtype=np.float32)
    for s in range(NCORES):
        o2 = res.results[s]["outT2"]  # [2, 4, BS]
        for c in range(C):
            out[s * BS:(s + 1) * BS, c] = o2[c % 2, c // 2]
    return out


# revision 21
# speedup vs baseline: 1.1849x; 1.1564x over previous
"""Trainium2 Bass kernel for nn_ConfounderStackLayers.

Computation (per batch row b, confounder c):
    h0 = relu(x @ W0[c].T + b0[c])        # [B, H0]
    h1 = relu(h0 @ W1[c].T + b1[c])       # [B, H1]
    out[b, c] = h1 @ W2[c, 0] + b2[c]     # scalar head

Sharding: data-parallel over batch across 8 cores (2048 rows each), weights
replicated, no collectives.

v2 design (from trace analysis of the f32r/alternating-evict baseline):
  - bf16 operands: enables FWL (fast weight load) so LDWEIGHTS hides under
    the 512-cycle matmul streaming; f32r paid a serial ~170ns LDWEIGHTS.
  - Single evict engine (ACT): every PSUM->SBUF evict (relu+bias) runs on
    the scalar engine.  All PE instruction deps then collapse onto the ACT
    sem and all ACT deps onto the PE sem, so the walrus one-wait-per-
    instruction budget is met with almost no "touch" scaffolding.
  - Layer 2 col-tiling: the per-(c,kt) scalar-head matmuls have M=1 but
    cost a full 512-row stream each.  They are batched per batch-chunk and
    issued as 4 concurrent column-tiles (tile_position via out base
    partition 32j), overlapping 4 chains of 4 accumulation matmuls ->
    ~4 slots instead of 16.  Group c=2j+cc lands on PSUM partition 32j+cc.
  - PE warmup: dummy matmuls run while the first DMA chunks land so the
    real stream starts at the warm (fast) PE clock.
  - DMA order: bias, then x/weights in consumption order, w2 last.
"""

import os
from contextlib import ExitStack

import numpy as np

import concourse.bass as bass
import concourse.mybir as mybir
import concourse.tile as tile
from concourse.tile_rust import add_dep_helper
from concourse.bass_utils import run_bass_kernel_spmd

NCORES = 8
B, C, D, H0, H1 = 16384, 8, 256, 512, 256
BS = B // NCORES          # 2048 batch rows per core
BC = 512                  # batch chunk (one psum bank of fp32)
NB = BS // BC             # 4
KT0, MT0 = D // 128, H0 // 128    # 2, 4
KT1, MT1 = H0 // 128, H1 // 128   # 4, 2
KT2 = H1 // 128                   # 2

WCOLS = KT0 * H0 + KT1 * H1       # per-c combined weight columns (w0 then w1)
W0C = KT0 * H0
W2_COLS = 4 * 2 * KT2 * 2         # [j, cc, kt, m] -> 32
B0_OFF, B1_OFF, B2_OFF = 0, C * MT0, C * MT0 + C * MT1
BIAS_COLS = B2_OFF + 1            # 49
NWARM = 8                         # PE warm-up dummy matmuls

MM_MODE = os.environ.get("KERNEL_MM_MODE", "bf16")

_CACHE = {}


def _build(mm: str) -> bass.Bass:
    f32 = mybir.dt.float32
    if mm == "bf16":
        wdt = mybir.dt.bfloat16
    elif mm == "f32r":
        wdt = mybir.dt.float32r
    else:
        wdt = f32
    relu = mybir.ActivationFunctionType.Relu
    copy_f = mybir.ActivationFunctionType.Copy

    nc = bass.Bass(trn_type="TRN2")
    xd = nc.dram_tensor("xd", [128, NB, KT0, BC], wdt, kind="ExternalInput")
    w2d = nc.dram_tensor("w2d", [128, W2_COLS], wdt, kind="ExternalInput")
    wcat = nc.dram_tensor("wcat", [C, 128, WCOLS], wdt, kind="ExternalInput")
    biasd = nc.dram_tensor("biasd", [128, BIAS_COLS], f32, kind="ExternalInput")
    outT2 = nc.dram_tensor("outT2", [2, 4, BS], f32, kind="ExternalOutput")

    with tile.TileContext(nc) as tc, ExitStack() as ctx:
        consts = ctx.enter_context(tc.tile_pool(name="consts", bufs=1))
        h0p = ctx.enter_context(tc.tile_pool(name="h0", bufs=3))
        h1p = ctx.enter_context(tc.tile_pool(name="h1", bufs=8))
        ps0p = ctx.enter_context(tc.tile_pool(name="ps0", bufs=4, space="PSUM"))
        ps1p = ctx.enter_context(tc.tile_pool(name="ps1", bufs=2, space="PSUM"))
        ps2p = ctx.enter_context(tc.tile_pool(name="ps2", bufs=1, space="PSUM"))
        pewp = ctx.enter_context(tc.tile_pool(name="pew", bufs=1, space="PSUM"))

        bias = consts.tile([128, BIAS_COLS], f32, tag="bias")
        xs = consts.tile([128, NB, KT0, BC], wdt, tag="xs")
        wts = [
            consts.tile([128, WCOLS], wdt, name=f"w_{c}", tag=f"w_{c}")
            for c in range(C)
        ]
        w2s = consts.tile([128, W2_COLS], wdt, tag="w2s")
        zeros = consts.tile([1, BC], wdt, tag="zeros")
        outt = consts.tile([128, NB, BC], f32, tag="outt")
        act_scr = consts.tile([1, 64], f32, tag="act_scr")
        dve_scr = consts.tile([1, 64], f32, tag="dve_scr")
        gp_scr = consts.tile([1, 8], f32, tag="gp_scr")
        pewarm = pewp.tile([128, BC], f32, tag="pewarm")

        mset = nc.gpsimd.memset(zeros[:, :], 0.0)

        # Input DMAs in consumption order on the HWDGE ring.
        funnel_deps = []
        funnel_deps.append(nc.sync.dma_start(out=bias, in_=biasd[:, :]))
        dma_order = [("x", 0), ("w0", 0), ("w1", 0), ("w0", 1), ("w1", 1),
                     ("x", 1), ("w0", 2), ("w1", 2), ("w0", 3), ("w1", 3),
                     ("x", 2), ("w0", 4), ("w1", 4), ("w0", 5), ("w1", 5),
                     ("x", 3), ("w0", 6), ("w1", 6), ("w0", 7), ("w1", 7)]
        for kind, i in dma_order:
            if kind == "x":
                funnel_deps.append(nc.sync.dma_start(out=xs[:, i], in_=xd[:, i]))
            elif kind == "w0":
                funnel_deps.append(
                    nc.sync.dma_start(out=wts[i][:, 0:W0C], in_=wcat[i][:, 0:W0C])
                )
            else:
                funnel_deps.append(
                    nc.sync.dma_start(
                        out=wts[i][:, W0C:WCOLS], in_=wcat[i][:, W0C:WCOLS]
                    )
                )
        funnel_deps.append(nc.sync.dma_start(out=w2s, in_=w2d[:, :]))

        def xs_ap(kt, bi):
            return xs[:, bi, kt, :]

        def w0_ap(c, kt, mt):
            lo = kt * H0 + mt * 128
            return wts[c][:, lo:lo + 128]

        def w1_ap(c, kt, mt):
            lo = W0C + kt * H1 + mt * 128
            return wts[c][:, lo:lo + 128]

        def b0_ap(c, mt):
            return bias[:, B0_OFF + c * MT0 + mt:B0_OFF + c * MT0 + mt + 1]

        def b1_ap(c, mt):
            return bias[:, B1_OFF + c * MT1 + mt:B1_OFF + c * MT1 + mt + 1]

        b2_ap = bias[:, B2_OFF:B2_OFF + 1]

        def w2_ap(c, kt):
            j, cc = c // 2, c % 2
            off = (j * 4 + cc * 2 + kt) * 2
            return w2s[:, off:off + 2]

        state = {"pe_prev": None, "pe_cnt": 0, "act_cnt": 0, "dve_cnt": 0,
                 "first_evict": {"act": True, "dve": True}}
        add_op = mybir.AluOpType.add
        max_op = mybir.AluOpType.max

        def act_touch(src_ap):
            j = state["act_cnt"]
            state["act_cnt"] += 1
            return nc.scalar.activation(act_scr[0:1, j:j + 1], src_ap, copy_f)

        def dve_touch(src_ap):
            j = state["dve_cnt"]
            state["dve_cnt"] += 1
            return nc.vector.tensor_copy(dve_scr[0:1, j:j + 1], src_ap)

        def pe_pin(inst):
            # no-sync scheduler edge chaining PE program order
            if state["pe_prev"] is not None:
                add_dep_helper(inst.ins, state["pe_prev"].ins, False, "pe order")
            state["pe_prev"] = inst
            return inst

        def pe_touch(col_ap):
            j = state["pe_cnt"]
            state["pe_cnt"] += 1
            if col_ap.dtype == mybir.dt.float32r:
                col_ap = col_ap.bitcast(mybir.dt.float32)
            t = nc.tensor.matmul(
                pewarm[0:1, j:j + 1], lhsT=col_ap, rhs=col_ap,
                start=True, stop=True,
            )
            return pe_pin(t)

        z_lhs = zeros[0:1, 0:128]
        z_rhs = zeros[0:1, 0:BC]
        if wdt == mybir.dt.float32r:
            z_lhs = z_lhs.bitcast(f32)
            z_rhs = z_rhs.bitcast(f32)

        # PE warm-up while the first DMAs land.
        for _ in range(NWARM):
            pe_pin(nc.tensor.matmul(
                pewarm[:, :], lhsT=z_lhs, rhs=z_rhs, start=True, stop=True))

        act_bias_touch = act_touch(bias[0:1, 0:1])
        dve_bias_touch = dve_touch(bias[0:1, 0:1])

        def evict(engine, dst_ap, src_ps, bias_ap, with_relu=True,
                  touch_first=False):
            # A slot-reusing evict would carry two waits (own-sem slot release
            # + PE psum); the touch absorbs the PE wait first.
            t = None
            if touch_first:
                t = act_touch(src_ps[0:1, 0:1]) if engine == "act" \
                    else dve_touch(src_ps[0:1, 0:1])
            if engine == "act":
                if with_relu:
                    e = nc.scalar.activation(dst_ap, src_ps, relu, bias=bias_ap)
                else:
                    e = nc.scalar.add(dst_ap, src_ps, bias_ap)
            else:
                e = nc.vector.tensor_scalar(
                    dst_ap, src_ps, bias_ap, 0.0, add_op, max_op)
            if engine == "dve":
                state["last_dve"] = e
            if t is not None:
                add_dep_helper(e.ins, t.ins, False, "touch before evict")
            if state["first_evict"][engine]:
                state["first_evict"][engine] = False
                bt = act_bias_touch if engine == "act" else dve_bias_touch
                add_dep_helper((t or e).ins, bt.ins, False, "bias touch first")
            return e

        last_act = None
        for bi in range(NB):
            ps2 = ps2p.tile([128, BC], f32)
            if bi == 0:
                # zero the full bank once so the out-evict never reads
                # uninitialized PSUM in the unused partition rows
                pe_pin(nc.tensor.matmul(
                    ps2[:, :], lhsT=z_lhs, rhs=z_rhs, start=True, stop=True))
            xt = pe_touch(xs[:, bi, 0, 0:1])
            h1s = []
            h0s = {}
            # Software pipeline: phase ph runs L0 of c=ph and L1 of c=ph-1,
            # so the ACT evicts of h0[c] get a full extra phase (~1.7us)
            # before the L1 matmuls consume them.
            for ph in range(C + 1):
                if ph < C:
                    c = ph
                    if bi == 0:
                        wt = pe_touch(wts[c][:, 0:1])
                    h0 = h0p.tile([128, KT1, BC], wdt)
                    h0s[c] = h0
                    for mt in range(MT0):
                        ps0 = ps0p.tile([128, BC], f32)
                        for kt in range(KT0):
                            mmi = pe_pin(nc.tensor.matmul(
                                ps0,
                                lhsT=w0_ap(c, kt, mt),
                                rhs=xs_ap(kt, bi),
                                start=(kt == 0),
                                stop=(kt == KT0 - 1),
                            ))
                            if bi == 0 and mt == 0 and kt == 0:
                                add_dep_helper(mmi.ins, wt.ins, False, "wt first")
                            if c == 0 and mt == 0 and kt == 0:
                                add_dep_helper(mmi.ins, xt.ins, False, "xt first")
                        evict("act", h0[:, mt, :], ps0, b0_ap(c, mt),
                              touch_first=(mt == 0))
                if ph >= 1:
                    c = ph - 1
                    h0 = h0s.pop(c)
                    if bi == 0:
                        wt1 = pe_touch(wts[c][:, W0C:W0C + 1])
                    # Absorb the ACT (h0 write) wait on PE so the first L1
                    # matmul carries only its ps1 slot-release (DVE) wait.
                    h0t = pe_touch(h0[0:1, 0, 0:1])
                    h1 = h1p.tile([128, KT2, BC], wdt)
                    h1s.append(h1)
                    for mt in range(MT1):
                        ps1 = ps1p.tile([128, BC], f32)
                        for kt in range(KT1):
                            mm1 = pe_pin(nc.tensor.matmul(
                                ps1,
                                lhsT=w1_ap(c, kt, mt),
                                rhs=h0[:, kt, :],
                                start=(kt == 0),
                                stop=(kt == KT1 - 1),
                            ))
                            if bi == 0 and mt == 0 and kt == 0:
                                add_dep_helper(mm1.ins, wt1.ins, False, "w1t first")
                            if mt == 0 and kt == 0:
                                add_dep_helper(mm1.ins, h0t.ins, False, "h0t first")
                        evict("dve", h1[:, mt, :], ps1, b1_ap(c, mt),
                              touch_first=(mt == 0))
            # Layer 2: batched col-tiled scalar heads.  Column group
            # j = c//2 accumulates c in {2j, 2j+1} into psum partitions
            # [32j : 32j+2); the 4 groups' chains overlap in the PE array.
            if bi == 0:
                w2t = pe_touch(w2s[:, 0:1])
            if bi >= 1:
                # Absorb the ps2 slot-release (ACT out-evict of bi-1) so the
                # first L2 matmul carries only its h1 (DVE) wait.
                pe_touch(outt[0:1, bi - 1, 0:1])
            for c in range(C):
                j, cc = c // 2, c % 2
                for kt in range(KT2):
                    mm2 = pe_pin(nc.tensor.matmul(
                        ps2[32 * j:32 * j + 2, :],
                        lhsT=w2_ap(c, kt),
                        rhs=h1s[c][:, kt, :],
                        start=(cc == 0 and kt == 0),
                        stop=(cc == 1 and kt == KT2 - 1),
                        tile_position=(0, 32 * j),
                    ))
                    if bi == 0 and c == 0 and kt == 0:
                        add_dep_helper(mm2.ins, w2t.ins, False, "w2t first")
            last_act = evict("act", outt[:, bi, :], ps2, b2_ap, with_relu=False)
            # gpsimd touch absorbs the ACT (out-evict) wait so the DMA
            # triggers carry only their SW-queue sem.
            gpt = nc.gpsimd.tensor_copy(
                gp_scr[0:1, bi:bi + 1], outt[0:1, bi, 0:1])
            for m in range(2):
                od = nc.gpsimd.dma_start(
                    out=outT2[m][:, bi * BC:(bi + 1) * BC],
                    in_=outt[m:97 + m:32, bi, :],
                )
                add_dep_helper(od.ins, gpt.ins, False, "gp touch before dma")
                funnel_deps.append(od)

        funnel_deps += [mset, last_act, state["last_dve"], state["pe_prev"]]
        for dep in funnel_deps:
            n = nc.sync.nop()
            add_dep_helper(n.ins, dep.ins, True, "drain funnel")
    return nc


def _np_wdt(mm: str):
    if mm == "bf16":
        import ml_dtypes

        return ml_dtypes.bfloat16
    return np.float32


def kernel(x, W0, b0, W1, b1, W2, b2, trace=False):
    mm = MM_MODE
    key = ("nc", mm)
    if key not in _CACHE:
        _CACHE[key] = _build(mm)
    nc = _CACHE[key]
    wnp = _np_wdt(mm)

    x = np.ascontiguousarray(np.asarray(x, dtype=np.float32))
    W0 = np.asarray(W0, dtype=np.float32)
    W1 = np.asarray(W1, dtype=np.float32)
    W2 = np.asarray(W2, dtype=np.float32)
    b0 = np.asarray(b0, dtype=np.float32)
    b1 = np.asarray(b1, dtype=np.float32)
    b2 = np.asarray(b2, dtype=np.float32)

    # Combined per-c weight block: [C, 128, KT0*H0 + KT1*H1] where
    # wcat[c, p, kt*H0 + h] = W0[c, h, kt*128+p] and
    # wcat[c, p, KT0*H0 + kt*H1 + o] = W1[c, o, kt*128+p].
    wcat = np.empty((C, 128, WCOLS), dtype=np.float32)
    w0v = wcat[:, :, :W0C].reshape(C, 128, KT0, H0)
    w0v[...] = W0.reshape(C, H0, KT0, 128).transpose(0, 3, 2, 1)
    w1v = wcat[:, :, W0C:].reshape(C, 128, KT1, H1)
    w1v[...] = W1.reshape(C, H1, KT1, 128).transpose(0, 3, 2, 1)
    wcat = np.ascontiguousarray(wcat).astype(wnp)

    # Layer-2 col-tiled lhsT tiles: for (c, kt) a [128, 2] tile at column
    # (j*4 + cc*2 + kt)*2, with the W2 slice in column m=cc, 0 in the other.
    w2part = np.zeros((128, W2_COLS), dtype=np.float32)
    for c in range(C):
        j, cc = c // 2, c % 2
        for kt in range(KT2):
            col = (j * 4 + cc * 2 + kt) * 2 + cc
            w2part[:, col] = W2[c, 0, kt * 128:(kt + 1) * 128]
    w2part = w2part.astype(wnp)

    biasd = np.zeros((128, BIAS_COLS), dtype=np.float32)
    biasd[:, B0_OFF:B0_OFF + C * MT0] = (
        b0.reshape(C, MT0, 128).transpose(2, 0, 1).reshape(128, C * MT0)
    )
    biasd[:, B1_OFF:B1_OFF + C * MT1] = (
        b1.reshape(C, MT1, 128).transpose(2, 0, 1).reshape(128, C * MT1)
    )
    for c in range(C):
        biasd[32 * (c // 2) + c % 2, B2_OFF] = b2[c]

    xTfull = np.ascontiguousarray(x.T)  # [D, B] fp32
    in_maps = []
    for s in range(NCORES):
        xsh = xTfull[:, s * BS:(s + 1) * BS]          # [D, BS]
        # xd[p, bi, kt, b] = x[s*BS + bi*BC + b, kt*128 + p]
        xdn = np.ascontiguousarray(
            xsh.reshape(KT0, 128, NB, BC).transpose(1, 2, 0, 3)
        ).astype(wnp)
        in_maps.append(
            {"xd": xdn, "w2d": w2part, "wcat": wcat, "biasd": biasd}
        )

    res = run_bass_kernel_spmd(
        nc, in_maps, core_ids=list(range(NCORES)), trace=trace
    )
    _CACHE["last_result"] = res

    out = np.empty((B, C), dtype=np.float32)
    for s in range(NCORES):
        o2 = res.results[s]["outT2"]  # [2, 4, BS]
        for c in range(C):
            out[s * BS:(s + 1) * BS, c] = o2[c % 2, c // 2]
    return out


# revision 26
# speedup vs baseline: 1.2596x; 1.0630x over previous
"""Trainium2 Bass kernel for nn_ConfounderStackLayers.

Computation (per batch row b, confounder c):
    h0 = relu(x @ W0[c].T + b0[c])        # [B, H0]
    h1 = relu(h0 @ W1[c].T + b1[c])       # [B, H1]
    out[b, c] = h1 @ W2[c, 0] + b2[c]     # scalar head

Sharding: data-parallel over batch across 8 cores (2048 rows each), weights
replicated, no collectives.

v2 design (from trace analysis of the f32r/alternating-evict baseline):
  - bf16 operands: enables FWL (fast weight load) so LDWEIGHTS hides under
    the 512-cycle matmul streaming; f32r paid a serial ~170ns LDWEIGHTS.
  - Single evict engine (ACT): every PSUM->SBUF evict (relu+bias) runs on
    the scalar engine.  All PE instruction deps then collapse onto the ACT
    sem and all ACT deps onto the PE sem, so the walrus one-wait-per-
    instruction budget is met with almost no "touch" scaffolding.
  - Layer 2 col-tiling: the per-(c,kt) scalar-head matmuls have M=1 but
    cost a full 512-row stream each.  They are batched per batch-chunk and
    issued as 4 concurrent column-tiles (tile_position via out base
    partition 32j), overlapping 4 chains of 4 accumulation matmuls ->
    ~4 slots instead of 16.  Group c=2j+cc lands on PSUM partition 32j+cc.
  - PE warmup: dummy matmuls run while the first DMA chunks land so the
    real stream starts at the warm (fast) PE clock.
  - DMA order: bias, then x/weights in consumption order, w2 last.
"""

import os
from contextlib import ExitStack

import numpy as np

import concourse.bass as bass
import concourse.mybir as mybir
import concourse.tile as tile
from concourse.tile_rust import add_dep_helper
from concourse.bass_utils import run_bass_kernel_spmd

NCORES = 8
B, C, D, H0, H1 = 16384, 8, 256, 512, 256
BS = B // NCORES          # 2048 batch rows per core
BC = 512                  # batch chunk (one psum bank of fp32)
NB = BS // BC             # 4
KT0, MT0 = D // 128, H0 // 128    # 2, 4
KT1, MT1 = H0 // 128, H1 // 128   # 4, 2
KT2 = H1 // 128                   # 2

WCOLS = KT0 * H0 + KT1 * H1       # per-c combined weight columns (w0 then w1)
W0C = KT0 * H0
W2_COLS = 4 * 2 * KT2 * 2         # [j, cc, kt, m] -> 32
B0_OFF, B1_OFF, B2_OFF = 0, C * MT0, C * MT0 + C * MT1
BIAS_COLS = B2_OFF + 1            # 49
NWARM = 8                         # PE warm-up dummy matmuls

MM_MODE = os.environ.get("KERNEL_MM_MODE", "bf16")

_CACHE = {}


def _build(mm: str) -> bass.Bass:
    f32 = mybir.dt.float32
    if mm == "bf16":
        wdt = mybir.dt.bfloat16
    elif mm == "f32r":
        wdt = mybir.dt.float32r
    else:
        wdt = f32
    relu = mybir.ActivationFunctionType.Relu
    copy_f = mybir.ActivationFunctionType.Copy

    nc = bass.Bass(trn_type="TRN2")
    xd = nc.dram_tensor("xd", [128, NB, KT0, BC], wdt, kind="ExternalInput")
    w2d = nc.dram_tensor("w2d", [128, W2_COLS], wdt, kind="ExternalInput")
    wcat = nc.dram_tensor("wcat", [C, 128, WCOLS], wdt, kind="ExternalInput")
    biasd = nc.dram_tensor("biasd", [128, BIAS_COLS], f32, kind="ExternalInput")
    outT2 = nc.dram_tensor("outT2", [2, 4, BS], f32, kind="ExternalOutput")

    with tile.TileContext(nc) as tc, ExitStack() as ctx:
        consts = ctx.enter_context(tc.tile_pool(name="consts", bufs=1))
        h0p = ctx.enter_context(tc.tile_pool(name="h0", bufs=3))
        h1p = ctx.enter_context(tc.tile_pool(name="h1", bufs=8))
        ps0p = ctx.enter_context(tc.tile_pool(name="ps0", bufs=4, space="PSUM"))
        ps1p = ctx.enter_context(tc.tile_pool(name="ps1", bufs=3, space="PSUM"))
        ps2p = ctx.enter_context(tc.tile_pool(name="ps2", bufs=1, space="PSUM"))

        bias = consts.tile([128, BIAS_COLS], f32, tag="bias")
        xs = consts.tile([128, NB, KT0, BC], wdt, tag="xs")
        wts = [
            consts.tile([128, WCOLS], wdt, name=f"w_{c}", tag=f"w_{c}")
            for c in range(C)
        ]
        w2s = consts.tile([128, W2_COLS], wdt, tag="w2s")
        zeros = consts.tile([1, BC], wdt, tag="zeros")
        outt = consts.tile([128, NB, BC], f32, tag="outt")
        act_scr = consts.tile([1, 64], f32, tag="act_scr")
        dve_scr = consts.tile([1, 64], f32, tag="dve_scr")
        gp_scr = consts.tile([1, 8], f32, tag="gp_scr")

        mset = nc.gpsimd.memset(zeros[:, :], 0.0)

        # Input DMAs in consumption order on the HWDGE ring.
        funnel_deps = []
        funnel_deps.append(nc.sync.dma_start(out=bias, in_=biasd[:, :]))
        dma_order = [("x", 0), ("w0", 0), ("w1", 0), ("w0", 1), ("w1", 1),
                     ("x", 1), ("w0", 2), ("w1", 2), ("w0", 3), ("w1", 3),
                     ("x", 2), ("w0", 4), ("w1", 4), ("w0", 5), ("w1", 5),
                     ("x", 3), ("w0", 6), ("w1", 6), ("w0", 7), ("w1", 7)]
        for kind, i in dma_order:
            if kind == "x":
                funnel_deps.append(nc.sync.dma_start(out=xs[:, i], in_=xd[:, i]))
            elif kind == "w0":
                funnel_deps.append(
                    nc.sync.dma_start(out=wts[i][:, 0:W0C], in_=wcat[i][:, 0:W0C])
                )
            else:
                funnel_deps.append(
                    nc.sync.dma_start(
                        out=wts[i][:, W0C:WCOLS], in_=wcat[i][:, W0C:WCOLS]
                    )
                )
        funnel_deps.append(nc.sync.dma_start(out=w2s, in_=w2d[:, :]))

        def xs_ap(kt, bi):
            return xs[:, bi, kt, :]

        def w0_ap(c, kt, mt):
            lo = kt * H0 + mt * 128
            return wts[c][:, lo:lo + 128]

        def w1_ap(c, kt, mt):
            lo = W0C + kt * H1 + mt * 128
            return wts[c][:, lo:lo + 128]

        def b0_ap(c, mt):
            return bias[:, B0_OFF + c * MT0 + mt:B0_OFF + c * MT0 + mt + 1]

        def b1_ap(c, mt):
            return bias[:, B1_OFF + c * MT1 + mt:B1_OFF + c * MT1 + mt + 1]

        b2_ap = bias[:, B2_OFF:B2_OFF + 1]

        def w2_ap(c, kt):
            j, cc = c // 2, c % 2
            off = (j * 4 + cc * 2 + kt) * 2
            return w2s[:, off:off + 2]

        state = {"pe_prev": None, "pe_cnt": 0, "act_cnt": 0, "dve_cnt": 0,
                 "first_evict": {"act": True, "dve": True}}
        add_op = mybir.AluOpType.add
        max_op = mybir.AluOpType.max

        def act_touch(src_ap):
            j = state["act_cnt"]
            state["act_cnt"] += 1
            return nc.scalar.activation(act_scr[0:1, j:j + 1], src_ap, copy_f)

        def dve_touch(src_ap):
            j = state["dve_cnt"]
            state["dve_cnt"] += 1
            return nc.vector.tensor_copy(dve_scr[0:1, j:j + 1], src_ap)

        def pe_pin(inst):
            # no-sync scheduler edge chaining PE program order
            if state["pe_prev"] is not None:
                add_dep_helper(inst.ins, state["pe_prev"].ins, False, "pe order")
            state["pe_prev"] = inst
            return inst

        def pe_touch(col_ap):
            # Standalone LDWEIGHTS: a PE instruction that reads the AP (so it
            # absorbs one sem wait) without touching PSUM.  The next real
            # matmul self-loads its own weights, so the clobber is harmless.
            state["pe_cnt"] += 1
            if col_ap.dtype in (mybir.dt.float32, mybir.dt.float32r):
                col_ap = col_ap.bitcast(mybir.dt.bfloat16)
            t = nc.tensor.ldweights(col_ap)
            return pe_pin(t)

        z_lhs = zeros[0:1, 0:128]
        z_rhs = zeros[0:1, 0:BC]
        if wdt == mybir.dt.float32r:
            z_lhs = z_lhs.bitcast(f32)
            z_rhs = z_rhs.bitcast(f32)

        # PE warm-up while the first DMAs land.
        pswarm = ps0p.tile([128, BC], f32, name="ps0")
        for _ in range(NWARM):
            pe_pin(nc.tensor.matmul(
                pswarm[:, :], lhsT=z_lhs, rhs=z_rhs, start=True, stop=True))

        act_bias_touch = act_touch(bias[0:1, 0:1])
        dve_bias_touch = dve_touch(bias[0:1, 0:1])

        def evict(engine, dst_ap, src_ps, bias_ap, with_relu=True,
                  touch_first=False):
            # A slot-reusing evict would carry two waits (own-sem slot release
            # + PE psum); the touch absorbs the PE wait first.
            t = None
            if touch_first:
                t = act_touch(src_ps[0:1, 0:1]) if engine == "act" \
                    else dve_touch(src_ps[0:1, 0:1])
            if engine == "act":
                if with_relu:
                    e = nc.scalar.activation(dst_ap, src_ps, relu, bias=bias_ap)
                else:
                    e = nc.scalar.add(dst_ap, src_ps, bias_ap)
            else:
                e = nc.vector.tensor_scalar(
                    dst_ap, src_ps, bias_ap, 0.0, add_op, max_op)
            if engine == "dve":
                state["last_dve"] = e
            if t is not None:
                add_dep_helper(e.ins, t.ins, False, "touch before evict")
            if state["first_evict"][engine]:
                state["first_evict"][engine] = False
                bt = act_bias_touch if engine == "act" else dve_bias_touch
                add_dep_helper((t or e).ins, bt.ins, False, "bias touch first")
            return e

        last_act = None
        for bi in range(NB):
            ps2 = ps2p.tile([128, BC], f32)
            if bi == 0:
                # zero the full bank once so the out-evict never reads
                # uninitialized PSUM in the unused partition rows
                pe_pin(nc.tensor.matmul(
                    ps2[:, :], lhsT=z_lhs, rhs=z_rhs, start=True, stop=True))
            xt = pe_touch(xs[:, bi, 0, 0:1])
            h1s = []
            h0s = {}
            # Software pipeline: phase ph runs L0 of c=ph and L1 of c=ph-1,
            # so the ACT evicts of h0[c] get a full extra phase (~1.7us)
            # before the L1 matmuls consume them.
            for ph in range(C + 1):
                if ph < C:
                    c = ph
                    if bi == 0:
                        wt = pe_touch(wts[c][:, 0:1])
                    h0 = h0p.tile([128, KT1, BC], wdt)
                    h0s[c] = h0
                    for mt in range(MT0):
                        ps0 = ps0p.tile([128, BC], f32)
                        for kt in range(KT0):
                            mmi = pe_pin(nc.tensor.matmul(
                                ps0,
                                lhsT=w0_ap(c, kt, mt),
                                rhs=xs_ap(kt, bi),
                                start=(kt == 0),
                                stop=(kt == KT0 - 1),
                            ))
                            if bi == 0 and mt == 0 and kt == 0:
                                add_dep_helper(mmi.ins, wt.ins, False, "wt first")
                            if c == 0 and mt == 0 and kt == 0:
                                add_dep_helper(mmi.ins, xt.ins, False, "xt first")
                        evict("act", h0[:, mt, :], ps0, b0_ap(c, mt),
                              touch_first=(mt == 0))
                if ph >= 1:
                    c = ph - 1
                    h0 = h0s.pop(c)
                    if bi == 0:
                        wt1 = pe_touch(wts[c][:, W0C:W0C + 1])
                    # Absorb the ACT (h0 write) wait on PE so the first L1
                    # matmul carries only its ps1 slot-release (DVE) wait.
                    h0t = pe_touch(h0[0:1, 0, 0:1])
                    h1 = h1p.tile([128, KT2, BC], wdt)
                    h1s.append(h1)
                    for mt in range(MT1):
                        ps1 = ps1p.tile([128, BC], f32)
                        for kt in range(KT1):
                            mm1 = pe_pin(nc.tensor.matmul(
                                ps1,
                                lhsT=w1_ap(c, kt, mt),
                                rhs=h0[:, kt, :],
                                start=(kt == 0),
                                stop=(kt == KT1 - 1),
                            ))
                            if bi == 0 and mt == 0 and kt == 0:
                                add_dep_helper(mm1.ins, wt1.ins, False, "w1t first")
                            if mt == 0 and kt == 0:
                                add_dep_helper(mm1.ins, h0t.ins, False, "h0t first")
                        evict("dve", h1[:, mt, :], ps1, b1_ap(c, mt),
                              touch_first=(mt == 0))
            # Layer 2: batched col-tiled scalar heads.  Column group
            # j = c//2 accumulates c in {2j, 2j+1} into psum partitions
            # [32j : 32j+2); the 4 groups' chains overlap in the PE array.
            if bi == 0:
                w2t = pe_touch(w2s[:, 0:1])
            if bi >= 1:
                # Absorb the ps2 slot-release (ACT out-evict of bi-1) so the
                # first L2 matmul carries only its h1 (DVE) wait.
                pe_touch(outt[0:1, bi - 1, 0:1])
            for c in range(C):
                j, cc = c // 2, c % 2
                for kt in range(KT2):
                    mm2 = pe_pin(nc.tensor.matmul(
                        ps2[32 * j:32 * j + 2, :],
                        lhsT=w2_ap(c, kt),
                        rhs=h1s[c][:, kt, :],
                        start=(cc == 0 and kt == 0),
                        stop=(cc == 1 and kt == KT2 - 1),
                        tile_position=(0, 32 * j),
                    ))
                    if bi == 0 and c == 0 and kt == 0:
                        add_dep_helper(mm2.ins, w2t.ins, False, "w2t first")
            last_act = evict("act", outt[:, bi, :], ps2, b2_ap, with_relu=False)
            # gpsimd touch absorbs the ACT (out-evict) wait so the DMA
            # triggers carry only their SW-queue sem.
            gpt = nc.gpsimd.tensor_copy(
                gp_scr[0:1, bi:bi + 1], outt[0:1, bi, 0:1])
            od0 = nc.gpsimd.dma_start(
                out=outT2[0][:, bi * BC:(bi + 1) * BC],
                in_=outt[0:97:32, bi, :],
            )
            add_dep_helper(od0.ins, gpt.ins, False, "gp touch before dma")
            od1 = nc.gpsimd.dma_start(
                out=outT2[1][:, bi * BC:(bi + 1) * BC],
                in_=outt[1:98:32, bi, :],
            )
            add_dep_helper(od1.ins, gpt.ins, False, "gp touch before dma")
            funnel_deps += [od0, od1]

        funnel_deps += [mset, last_act, state["last_dve"], state["pe_prev"]]
        for dep in funnel_deps:
            n = nc.sync.nop()
            add_dep_helper(n.ins, dep.ins, True, "drain funnel")
    return nc


def _np_wdt(mm: str):
    if mm == "bf16":
        import ml_dtypes

        return ml_dtypes.bfloat16
    return np.float32


def kernel(x, W0, b0, W1, b1, W2, b2, trace=False):
    mm = MM_MODE
    key = ("nc", mm)
    if key not in _CACHE:
        _CACHE[key] = _build(mm)
    nc = _CACHE[key]
    wnp = _np_wdt(mm)

    x = np.ascontiguousarray(np.asarray(x, dtype=np.float32))
    W0 = np.asarray(W0, dtype=np.float32)
    W1 = np.asarray(W1, dtype=np.float32)
    W2 = np.asarray(W2, dtype=np.float32)
    b0 = np.asarray(b0, dtype=np.float32)
    b1 = np.asarray(b1, dtype=np.float32)
    b2 = np.asarray(b2, dtype=np.float32)

    # Combined per-c weight block: [C, 128, KT0*H0 + KT1*H1] where
    # wcat[c, p, kt*H0 + h] = W0[c, h, kt*128+p] and
    # wcat[c, p, KT0*H0 + kt*H1 + o] = W1[c, o, kt*128+p].
    wcat = np.empty((C, 128, WCOLS), dtype=np.float32)
    w0v = wcat[:, :, :W0C].reshape(C, 128, KT0, H0)
    w0v[...] = W0.reshape(C, H0, KT0, 128).transpose(0, 3, 2, 1)
    w1v = wcat[:, :, W0C:].reshape(C, 128, KT1, H1)
    w1v[...] = W1.reshape(C, H1, KT1, 128).transpose(0, 3, 2, 1)
    wcat = np.ascontiguousarray(wcat).astype(wnp)

    # Layer-2 col-tiled lhsT tiles: for (c, kt) a [128, 2] tile at column
    # (j*4 + cc*2 + kt)*2, with the W2 slice in column m=cc, 0 in the other.
    w2part = np.zeros((128, W2_COLS), dtype=np.float32)
    for c in range(C):
        j, cc = c // 2, c % 2
        for kt in range(KT2):
            col = (j * 4 + cc * 2 + kt) * 2 + cc
            w2part[:, col] = W2[c, 0, kt * 128:(kt + 1) * 128]
    w2part = w2part.astype(wnp)

    biasd = np.zeros((128, BIAS_COLS), dtype=np.float32)
    biasd[:, B0_OFF:B0_OFF + C * MT0] = (
        b0.reshape(C, MT0, 128).transpose(2, 0, 1).reshape(128, C * MT0)
    )
    biasd[:, B1_OFF:B1_OFF + C * MT1] = (
        b1.reshape(C, MT1, 128).transpose(2, 0, 1).reshape(128, C * MT1)
    )
    for c in range(C):
        biasd[32 * (c // 2) + c % 2, B2_OFF] = b2[c]

    xTfull = np.ascontiguousarray(x.T)  # [D, B] fp32
    in_maps = []
    for s in range(NCORES):
        xsh = xTfull[:, s * BS:(s + 1) * BS]          # [D, BS]
        # xd[p, bi, kt, b] = x[s*BS + bi*BC + b, kt*128 + p]
        xdn = np.ascontiguousarray(
            xsh.reshape(KT0, 128, NB, BC).transpose(1, 2, 0, 3)
        ).astype(wnp)
        in_maps.append(
            {"xd": xdn, "w2d": w2part, "wcat": wcat, "biasd": biasd}
        )

    res = run_bass_kernel_spmd(
        nc, in_maps, core_ids=list(range(NCORES)), trace=trace
    )
    _CACHE["last_result"] = res

    out = np.empty((B, C), dtype=np.float32)
    for s in range(NCORES):
        o2 = res.results[s]["outT2"]  # [2, 4, BS]
        for c in range(C):
            out[s * BS:(s + 1) * BS, c] = o2[c % 2, c // 2]
    return out


# revision 28
# speedup vs baseline: 1.2735x; 1.0110x over previous
"""Trainium2 Bass kernel for nn_ConfounderStackLayers.

Computation (per batch row b, confounder c):
    h0 = relu(x @ W0[c].T + b0[c])        # [B, H0]
    h1 = relu(h0 @ W1[c].T + b1[c])       # [B, H1]
    out[b, c] = h1 @ W2[c, 0] + b2[c]     # scalar head

Sharding: data-parallel over batch across 8 cores (2048 rows each), weights
replicated, no collectives.

v2 design (from trace analysis of the f32r/alternating-evict baseline):
  - bf16 operands: enables FWL (fast weight load) so LDWEIGHTS hides under
    the 512-cycle matmul streaming; f32r paid a serial ~170ns LDWEIGHTS.
  - Single evict engine (ACT): every PSUM->SBUF evict (relu+bias) runs on
    the scalar engine.  All PE instruction deps then collapse onto the ACT
    sem and all ACT deps onto the PE sem, so the walrus one-wait-per-
    instruction budget is met with almost no "touch" scaffolding.
  - Layer 2 col-tiling: the per-(c,kt) scalar-head matmuls have M=1 but
    cost a full 512-row stream each.  They are batched per batch-chunk and
    issued as 4 concurrent column-tiles (tile_position via out base
    partition 32j), overlapping 4 chains of 4 accumulation matmuls ->
    ~4 slots instead of 16.  Group c=2j+cc lands on PSUM partition 32j+cc.
  - PE warmup: dummy matmuls run while the first DMA chunks land so the
    real stream starts at the warm (fast) PE clock.
  - DMA order: bias, then x/weights in consumption order, w2 last.
"""

import os
from contextlib import ExitStack

import numpy as np

import concourse.bass as bass
import concourse.mybir as mybir
import concourse.tile as tile
from concourse.tile_rust import add_dep_helper
from concourse.bass_utils import run_bass_kernel_spmd

NCORES = 8
B, C, D, H0, H1 = 16384, 8, 256, 512, 256
BS = B // NCORES          # 2048 batch rows per core
BC = 512                  # batch chunk (one psum bank of fp32)
NB = BS // BC             # 4
KT0, MT0 = D // 128, H0 // 128    # 2, 4
KT1, MT1 = H0 // 128, H1 // 128   # 4, 2
KT2 = H1 // 128                   # 2

WCOLS = KT0 * H0 + KT1 * H1       # per-c combined weight columns (w0 then w1)
W0C = KT0 * H0
W2_COLS = 4 * 2 * KT2 * 2         # [j, cc, kt, m] -> 32
B0_OFF, B1_OFF, B2_OFF = 0, C * MT0, C * MT0 + C * MT1
BIAS_COLS = B2_OFF + 1            # 49
NWARM = 12                        # PE warm-up dummy matmuls

MM_MODE = os.environ.get("KERNEL_MM_MODE", "bf16")

_CACHE = {}


def _build(mm: str) -> bass.Bass:
    f32 = mybir.dt.float32
    if mm == "bf16":
        wdt = mybir.dt.bfloat16
    elif mm == "f32r":
        wdt = mybir.dt.float32r
    else:
        wdt = f32
    relu = mybir.ActivationFunctionType.Relu
    copy_f = mybir.ActivationFunctionType.Copy

    nc = bass.Bass(trn_type="TRN2")
    xd = nc.dram_tensor("xd", [128, NB, KT0, BC], wdt, kind="ExternalInput")
    w2d = nc.dram_tensor("w2d", [128, W2_COLS], wdt, kind="ExternalInput")
    wcat = nc.dram_tensor("wcat", [C, 128, WCOLS], wdt, kind="ExternalInput")
    biasd = nc.dram_tensor("biasd", [128, BIAS_COLS], f32, kind="ExternalInput")
    outT2 = nc.dram_tensor("outT2", [2, 4, BS], f32, kind="ExternalOutput")

    with tile.TileContext(nc) as tc, ExitStack() as ctx:
        consts = ctx.enter_context(tc.tile_pool(name="consts", bufs=1))
        h0p = ctx.enter_context(tc.tile_pool(name="h0", bufs=3))
        h1p = ctx.enter_context(tc.tile_pool(name="h1", bufs=8))
        ps0p = ctx.enter_context(tc.tile_pool(name="ps0", bufs=4, space="PSUM"))
        ps1p = ctx.enter_context(tc.tile_pool(name="ps1", bufs=3, space="PSUM"))
        ps2p = ctx.enter_context(tc.tile_pool(name="ps2", bufs=1, space="PSUM"))

        bias = consts.tile([128, BIAS_COLS], f32, tag="bias")
        xs = consts.tile([128, NB, KT0, BC], wdt, tag="xs")
        wts = [
            consts.tile([128, WCOLS], wdt, name=f"w_{c}", tag=f"w_{c}")
            for c in range(C)
        ]
        w2s = consts.tile([128, W2_COLS], wdt, tag="w2s")
        zeros = consts.tile([1, BC], wdt, tag="zeros")
        outt = consts.tile([128, NB, BC], f32, tag="outt")
        act_scr = consts.tile([1, 64], f32, tag="act_scr")
        dve_scr = consts.tile([1, 64], f32, tag="dve_scr")
        gp_scr = consts.tile([1, 8], f32, tag="gp_scr")

        mset = nc.gpsimd.memset(zeros[:, :], 0.0)

        # Input DMAs in consumption order on the HWDGE ring.
        funnel_deps = []
        funnel_deps.append(nc.sync.dma_start(out=xs[:, 0], in_=xd[:, 0]))
        funnel_deps.append(nc.sync.dma_start(out=bias, in_=biasd[:, :]))
        dma_order = [("w0", 0), ("w1", 0), ("w0", 1), ("w1", 1),
                     ("x", 1), ("w0", 2), ("w1", 2), ("w0", 3), ("w1", 3),
                     ("x", 2), ("w0", 4), ("w1", 4), ("w0", 5), ("w1", 5),
                     ("x", 3), ("w0", 6), ("w1", 6), ("w0", 7), ("w1", 7)]
        for kind, i in dma_order:
            if kind == "x":
                funnel_deps.append(nc.sync.dma_start(out=xs[:, i], in_=xd[:, i]))
            elif kind == "w0":
                funnel_deps.append(
                    nc.sync.dma_start(out=wts[i][:, 0:W0C], in_=wcat[i][:, 0:W0C])
                )
            else:
                funnel_deps.append(
                    nc.sync.dma_start(
                        out=wts[i][:, W0C:WCOLS], in_=wcat[i][:, W0C:WCOLS]
                    )
                )
        funnel_deps.append(nc.sync.dma_start(out=w2s, in_=w2d[:, :]))

        def xs_ap(kt, bi):
            return xs[:, bi, kt, :]

        def w0_ap(c, kt, mt):
            lo = kt * H0 + mt * 128
            return wts[c][:, lo:lo + 128]

        def w1_ap(c, kt, mt):
            lo = W0C + kt * H1 + mt * 128
            return wts[c][:, lo:lo + 128]

        def b0_ap(c, mt):
            return bias[:, B0_OFF + c * MT0 + mt:B0_OFF + c * MT0 + mt + 1]

        def b1_ap(c, mt):
            return bias[:, B1_OFF + c * MT1 + mt:B1_OFF + c * MT1 + mt + 1]

        b2_ap = bias[:, B2_OFF:B2_OFF + 1]

        def w2_ap(c, kt):
            j, cc = c // 2, c % 2
            off = (j * 4 + cc * 2 + kt) * 2
            return w2s[:, off:off + 2]

        state = {"pe_prev": None, "pe_cnt": 0, "act_cnt": 0, "dve_cnt": 0,
                 "first_evict": {"act": True, "dve": True}}
        add_op = mybir.AluOpType.add
        max_op = mybir.AluOpType.max

        def act_touch(src_ap):
            j = state["act_cnt"]
            state["act_cnt"] += 1
            return nc.scalar.activation(act_scr[0:1, j:j + 1], src_ap, copy_f)

        def dve_touch(src_ap):
            j = state["dve_cnt"]
            state["dve_cnt"] += 1
            return nc.vector.tensor_copy(dve_scr[0:1, j:j + 1], src_ap)

        def pe_pin(inst):
            # no-sync scheduler edge chaining PE program order
            if state["pe_prev"] is not None:
                add_dep_helper(inst.ins, state["pe_prev"].ins, False, "pe order")
            state["pe_prev"] = inst
            return inst

        def pe_touch(col_ap):
            # Standalone LDWEIGHTS: a PE instruction that reads the AP (so it
            # absorbs one sem wait) without touching PSUM.  The next real
            # matmul self-loads its own weights, so the clobber is harmless.
            state["pe_cnt"] += 1
            if col_ap.dtype in (mybir.dt.float32, mybir.dt.float32r):
                col_ap = col_ap.bitcast(mybir.dt.bfloat16)
            t = nc.tensor.ldweights(col_ap)
            return pe_pin(t)

        z_lhs = zeros[0:1, 0:128]
        z_rhs = zeros[0:1, 0:BC]
        if wdt == mybir.dt.float32r:
            z_lhs = z_lhs.bitcast(f32)
            z_rhs = z_rhs.bitcast(f32)

        # PE warm-up while the first DMAs land.
        pswarm = ps0p.tile([128, BC], f32, name="ps0")
        for _ in range(NWARM):
            pe_pin(nc.tensor.matmul(
                pswarm[:, :], lhsT=z_lhs, rhs=z_rhs, start=True, stop=True))

        act_bias_touch = act_touch(bias[0:1, 0:1])
        dve_bias_touch = dve_touch(bias[0:1, 0:1])

        def evict(engine, dst_ap, src_ps, bias_ap, with_relu=True,
                  touch_first=False):
            # A slot-reusing evict would carry two waits (own-sem slot release
            # + PE psum); the touch absorbs the PE wait first.
            t = None
            if touch_first:
                t = act_touch(src_ps[0:1, 0:1]) if engine == "act" \
                    else dve_touch(src_ps[0:1, 0:1])
            if engine == "act":
                if with_relu:
                    e = nc.scalar.activation(dst_ap, src_ps, relu, bias=bias_ap)
                else:
                    e = nc.scalar.add(dst_ap, src_ps, bias_ap)
            else:
                e = nc.vector.tensor_scalar(
                    dst_ap, src_ps, bias_ap, 0.0, add_op, max_op)
            if engine == "dve":
                state["last_dve"] = e
            if t is not None:
                add_dep_helper(e.ins, t.ins, False, "touch before evict")
            if state["first_evict"][engine]:
                state["first_evict"][engine] = False
                bt = act_bias_touch if engine == "act" else dve_bias_touch
                add_dep_helper((t or e).ins, bt.ins, False, "bias touch first")
            return e

        last_act = None
        pending = None  # deferred (bi, ps2, h1s) layer-2 batch

        def emit_l2(pbi, ps2, h1s):
            nonlocal last_act
            if pbi == 0:
                w2t = pe_touch(w2s[:, 0:1])
            if pbi >= 1:
                # Absorb the ps2 WAR (ACT out-evict of pbi-1) so the first
                # L2 matmul carries only its h1 (DVE) wait.
                pe_touch(outt[0:1, pbi - 1, 0:1])
            for c in range(C):
                j, cc = c // 2, c % 2
                for kt in range(KT2):
                    mm2 = pe_pin(nc.tensor.matmul(
                        ps2[32 * j:32 * j + 2, :],
                        lhsT=w2_ap(c, kt),
                        rhs=h1s[c][:, kt, :],
                        start=(cc == 0 and kt == 0),
                        stop=(cc == 1 and kt == KT2 - 1),
                        tile_position=(0, 32 * j),
                    ))
                    if pbi == 0 and c == 0 and kt == 0:
                        add_dep_helper(mm2.ins, w2t.ins, False, "w2t first")
            last_act = evict("act", outt[:, pbi, :], ps2, b2_ap,
                             with_relu=False)
            gpt = nc.gpsimd.tensor_copy(
                gp_scr[0:1, pbi:pbi + 1], outt[0:1, pbi, 0:1])
            od0 = nc.gpsimd.dma_start(
                out=outT2[0][:, pbi * BC:(pbi + 1) * BC],
                in_=outt[0:97:32, pbi, :],
            )
            add_dep_helper(od0.ins, gpt.ins, False, "gp touch before dma")
            od1 = nc.gpsimd.dma_start(
                out=outT2[1][:, pbi * BC:(pbi + 1) * BC],
                in_=outt[1:98:32, pbi, :],
            )
            add_dep_helper(od1.ins, gpt.ins, False, "gp touch before dma")
            funnel_deps.extend([od0, od1])

        for bi in range(NB):
            ps2 = ps2p.tile([128, BC], f32)
            if bi == 0:
                # zero the full bank once so the out-evict never reads
                # uninitialized PSUM in the unused partition rows
                pe_pin(nc.tensor.matmul(
                    ps2[:, :], lhsT=z_lhs, rhs=z_rhs, start=True, stop=True))
            xt = pe_touch(xs[:, bi, 0, 0:1])
            h1s = []
            h0s = {}
            # Software pipeline: phase ph runs L0 of c=ph and L1 of c=ph-1,
            # so the ACT evicts of h0[c] get a full extra phase (~1.7us)
            # before the L1 matmuls consume them.
            for ph in range(C + 1):
                if ph == 1 and pending is not None:
                    # Deferred layer-2 batch of the previous bi: running it
                    # after this bi's first L0 block hides the h1(c7) evict
                    # latency behind ~1.7us of layer-0 matmuls.
                    emit_l2(*pending)
                    pending = None
                if ph < C:
                    c = ph
                    if bi == 0:
                        wt = pe_touch(wts[c][:, 0:1])
                    h0 = h0p.tile([128, KT1, BC], wdt)
                    h0s[c] = h0
                    for mt in range(MT0):
                        ps0 = ps0p.tile([128, BC], f32)
                        for kt in range(KT0):
                            mmi = pe_pin(nc.tensor.matmul(
                                ps0,
                                lhsT=w0_ap(c, kt, mt),
                                rhs=xs_ap(kt, bi),
                                start=(kt == 0),
                                stop=(kt == KT0 - 1),
                            ))
                            if bi == 0 and mt == 0 and kt == 0:
                                add_dep_helper(mmi.ins, wt.ins, False, "wt first")
                            if c == 0 and mt == 0 and kt == 0:
                                add_dep_helper(mmi.ins, xt.ins, False, "xt first")
                        evict("act", h0[:, mt, :], ps0, b0_ap(c, mt),
                              touch_first=(mt == 0))
                if ph >= 1:
                    c = ph - 1
                    h0 = h0s.pop(c)
                    if bi == 0:
                        wt1 = pe_touch(wts[c][:, W0C:W0C + 1])
                    # Absorb the ACT (h0 write) wait on PE so the first L1
                    # matmul carries only its ps1 slot-release (DVE) wait.
                    h0t = pe_touch(h0[0:1, 0, 0:1])
                    h1 = h1p.tile([128, KT2, BC], wdt)
                    h1s.append(h1)
                    for mt in range(MT1):
                        ps1 = ps1p.tile([128, BC], f32)
                        for kt in range(KT1):
                            mm1 = pe_pin(nc.tensor.matmul(
                                ps1,
                                lhsT=w1_ap(c, kt, mt),
                                rhs=h0[:, kt, :],
                                start=(kt == 0),
                                stop=(kt == KT1 - 1),
                            ))
                            if bi == 0 and mt == 0 and kt == 0:
                                add_dep_helper(mm1.ins, wt1.ins, False, "w1t first")
                            if mt == 0 and kt == 0:
                                add_dep_helper(mm1.ins, h0t.ins, False, "h0t first")
                        evict("dve", h1[:, mt, :], ps1, b1_ap(c, mt),
                              touch_first=(mt == 0))
            pending = (bi, ps2, h1s)

        emit_l2(*pending)
        funnel_deps += [mset, last_act, state["last_dve"], state["pe_prev"]]
        for dep in funnel_deps:
            n = nc.sync.nop()
            add_dep_helper(n.ins, dep.ins, True, "drain funnel")
    return nc


def _np_wdt(mm: str):
    if mm == "bf16":
        import ml_dtypes

        return ml_dtypes.bfloat16
    return np.float32


def kernel(x, W0, b0, W1, b1, W2, b2, trace=False):
    mm = MM_MODE
    key = ("nc", mm)
    if key not in _CACHE:
        _CACHE[key] = _build(mm)
    nc = _CACHE[key]
    wnp = _np_wdt(mm)

    x = np.ascontiguousarray(np.asarray(x, dtype=np.float32))
    W0 = np.asarray(W0, dtype=np.float32)
    W1 = np.asarray(W1, dtype=np.float32)
    W2 = np.asarray(W2, dtype=np.float32)
    b0 = np.asarray(b0, dtype=np.float32)
    b1 = np.asarray(b1, dtype=np.float32)
    b2 = np.asarray(b2, dtype=np.float32)

    # Combined per-c weight block: [C, 128, KT0*H0 + KT1*H1] where
    # wcat[c, p, kt*H0 + h] = W0[c, h, kt*128+p] and
    # wcat[c, p, KT0*H0 + kt*H1 + o] = W1[c, o, kt*128+p].
    wcat = np.empty((C, 128, WCOLS), dtype=np.float32)
    w0v = wcat[:, :, :W0C].reshape(C, 128, KT0, H0)
    w0v[...] = W0.reshape(C, H0, KT0, 128).transpose(0, 3, 2, 1)
    w1v = wcat[:, :, W0C:].reshape(C, 128, KT1, H1)
    w1v[...] = W1.reshape(C, H1, KT1, 128).transpose(0, 3, 2, 1)
    wcat = np.ascontiguousarray(wcat).astype(wnp)

    # Layer-2 col-tiled lhsT tiles: for (c, kt) a [128, 2] tile at column
    # (j*4 + cc*2 + kt)*2, with the W2 slice in column m=cc, 0 in the other.
    w2part = np.zeros((128, W2_COLS), dtype=np.float32)
    for c in range(C):
        j, cc = c // 2, c % 2
        for kt in range(KT2):
            col = (j * 4 + cc * 2 + kt) * 2 + cc
            w2part[:, col] = W2[c, 0, kt * 128:(kt + 1) * 128]
    w2part = w2part.astype(wnp)

    biasd = np.zeros((128, BIAS_COLS), dtype=np.float32)
    biasd[:, B0_OFF:B0_OFF + C * MT0] = (
        b0.reshape(C, MT0, 128).transpose(2, 0, 1).reshape(128, C * MT0)
    )
    biasd[:, B1_OFF:B1_OFF + C * MT1] = (
        b1.reshape(C, MT1, 128).transpose(2, 0, 1).reshape(128, C * MT1)
    )
    for c in range(C):
        biasd[32 * (c // 2) + c % 2, B2_OFF] = b2[c]

    xTfull = np.ascontiguousarray(x.T)  # [D, B] fp32
    in_maps = []
    for s in range(NCORES):
        xsh = xTfull[:, s * BS:(s + 1) * BS]          # [D, BS]
        # xd[p, bi, kt, b] = x[s*BS + bi*BC + b, kt*128 + p]
        xdn = np.ascontiguousarray(
            xsh.reshape(KT0, 128, NB, BC).transpose(1, 2, 0, 3)
        ).astype(wnp)
        in_maps.append(
            {"xd": xdn, "w2d": w2part, "wcat": wcat, "biasd": biasd}
        )

    res = run_bass_kernel_spmd(
        nc, in_maps, core_ids=list(range(NCORES)), trace=trace
    )
    _CACHE["last_result"] = res

    out = np.empty((B, C), dtype=np.float32)
    for s in range(NCORES):
        o2 = res.results[s]["outT2"]  # [2, 4, BS]
        for c in range(C):
            out[s * BS:(s + 1) * BS, c] = o2[c % 2, c // 2]
    return out


# revision 29
# speedup vs baseline: 1.2950x; 1.0168x over previous
"""Trainium2 Bass kernel for nn_ConfounderStackLayers.

Computation (per batch row b, confounder c):
    h0 = relu(x @ W0[c].T + b0[c])        # [B, H0]
    h1 = relu(h0 @ W1[c].T + b1[c])       # [B, H1]
    out[b, c] = h1 @ W2[c, 0] + b2[c]     # scalar head

Sharding: data-parallel over batch across 8 cores (2048 rows each), weights
replicated, no collectives.

v2 design (from trace analysis of the f32r/alternating-evict baseline):
  - bf16 operands: enables FWL (fast weight load) so LDWEIGHTS hides under
    the 512-cycle matmul streaming; f32r paid a serial ~170ns LDWEIGHTS.
  - Single evict engine (ACT): every PSUM->SBUF evict (relu+bias) runs on
    the scalar engine.  All PE instruction deps then collapse onto the ACT
    sem and all ACT deps onto the PE sem, so the walrus one-wait-per-
    instruction budget is met with almost no "touch" scaffolding.
  - Layer 2 col-tiling: the per-(c,kt) scalar-head matmuls have M=1 but
    cost a full 512-row stream each.  They are batched per batch-chunk and
    issued as 4 concurrent column-tiles (tile_position via out base
    partition 32j), overlapping 4 chains of 4 accumulation matmuls ->
    ~4 slots instead of 16.  Group c=2j+cc lands on PSUM partition 32j+cc.
  - PE warmup: dummy matmuls run while the first DMA chunks land so the
    real stream starts at the warm (fast) PE clock.
  - DMA order: bias, then x/weights in consumption order, w2 last.
"""

import os
from contextlib import ExitStack

import numpy as np

import concourse.bass as bass
import concourse.mybir as mybir
import concourse.tile as tile
from concourse.tile_rust import add_dep_helper
from concourse.bass_utils import run_bass_kernel_spmd

NCORES = 8
B, C, D, H0, H1 = 16384, 8, 256, 512, 256
BS = B // NCORES          # 2048 batch rows per core
BC = 512                  # batch chunk (one psum bank of fp32)
NB = BS // BC             # 4
KT0, MT0 = D // 128, H0 // 128    # 2, 4
KT1, MT1 = H0 // 128, H1 // 128   # 4, 2
KT2 = H1 // 128                   # 2

WCOLS = KT0 * H0 + KT1 * H1       # per-c combined weight columns (w0 then w1)
W0C = KT0 * H0
W2_COLS = 4 * 2 * KT2 * 2         # [j, cc, kt, m] -> 32
B0_OFF, B1_OFF, B2_OFF = 0, C * MT0, C * MT0 + C * MT1
BIAS_COLS = B2_OFF + 1            # 49
NWARM = 12                        # PE warm-up dummy matmuls

MM_MODE = os.environ.get("KERNEL_MM_MODE", "bf16")

_CACHE = {}


def _build(mm: str) -> bass.Bass:
    f32 = mybir.dt.float32
    if mm == "bf16":
        wdt = mybir.dt.bfloat16
    elif mm == "f32r":
        wdt = mybir.dt.float32r
    else:
        wdt = f32
    relu = mybir.ActivationFunctionType.Relu
    copy_f = mybir.ActivationFunctionType.Copy

    nc = bass.Bass(trn_type="TRN2")
    xd = nc.dram_tensor("xd", [128, NB, KT0, BC], wdt, kind="ExternalInput")
    w2d = nc.dram_tensor("w2d", [128, W2_COLS], wdt, kind="ExternalInput")
    wcat = nc.dram_tensor("wcat", [C, 128, WCOLS], wdt, kind="ExternalInput")
    biasd = nc.dram_tensor("biasd", [128, BIAS_COLS], f32, kind="ExternalInput")
    outT2 = nc.dram_tensor("outT2", [2, 4, BS], f32, kind="ExternalOutput")

    with tile.TileContext(nc) as tc, ExitStack() as ctx:
        consts = ctx.enter_context(tc.tile_pool(name="consts", bufs=1))
        h0p = ctx.enter_context(tc.tile_pool(name="h0", bufs=3))
        h1p = ctx.enter_context(tc.tile_pool(name="h1", bufs=8))
        ps0p = ctx.enter_context(tc.tile_pool(name="ps0", bufs=4, space="PSUM"))
        ps1p = ctx.enter_context(tc.tile_pool(name="ps1", bufs=3, space="PSUM"))
        ps2p = ctx.enter_context(tc.tile_pool(name="ps2", bufs=1, space="PSUM"))

        bias = consts.tile([128, BIAS_COLS], f32, tag="bias")
        xs = consts.tile([128, NB, KT0, BC], wdt, tag="xs")
        wts = [
            consts.tile([128, WCOLS], wdt, name=f"w_{c}", tag=f"w_{c}")
            for c in range(C)
        ]
        w2s = consts.tile([128, W2_COLS], wdt, tag="w2s")
        zeros = consts.tile([128, 128 + BC], wdt, tag="zeros")
        outt = consts.tile([128, NB, BC], f32, tag="outt")
        act_scr = consts.tile([1, 64], f32, tag="act_scr")
        dve_scr = consts.tile([1, 64], f32, tag="dve_scr")
        gp_scr = consts.tile([1, 8], f32, tag="gp_scr")

        mset = nc.vector.memset(zeros[:, :], 0.0)

        # Input DMAs in consumption order on the HWDGE ring.
        funnel_deps = []
        funnel_deps.append(nc.sync.dma_start(out=xs[:, 0], in_=xd[:, 0]))
        funnel_deps.append(nc.sync.dma_start(out=bias, in_=biasd[:, :]))
        dma_order = [("w0", 0), ("w1", 0), ("w0", 1), ("w1", 1),
                     ("x", 1), ("w0", 2), ("w1", 2), ("w0", 3), ("w1", 3),
                     ("x", 2), ("w0", 4), ("w1", 4), ("w0", 5), ("w1", 5),
                     ("x", 3), ("w0", 6), ("w1", 6), ("w0", 7), ("w1", 7)]
        for kind, i in dma_order:
            if kind == "x":
                funnel_deps.append(nc.sync.dma_start(out=xs[:, i], in_=xd[:, i]))
            elif kind == "w0":
                funnel_deps.append(
                    nc.sync.dma_start(out=wts[i][:, 0:W0C], in_=wcat[i][:, 0:W0C])
                )
            else:
                funnel_deps.append(
                    nc.sync.dma_start(
                        out=wts[i][:, W0C:WCOLS], in_=wcat[i][:, W0C:WCOLS]
                    )
                )
        funnel_deps.append(nc.sync.dma_start(out=w2s, in_=w2d[:, :]))

        def xs_ap(kt, bi):
            return xs[:, bi, kt, :]

        def w0_ap(c, kt, mt):
            lo = kt * H0 + mt * 128
            return wts[c][:, lo:lo + 128]

        def w1_ap(c, kt, mt):
            lo = W0C + kt * H1 + mt * 128
            return wts[c][:, lo:lo + 128]

        def b0_ap(c, mt):
            return bias[:, B0_OFF + c * MT0 + mt:B0_OFF + c * MT0 + mt + 1]

        def b1_ap(c, mt):
            return bias[:, B1_OFF + c * MT1 + mt:B1_OFF + c * MT1 + mt + 1]

        b2_ap = bias[:, B2_OFF:B2_OFF + 1]

        def w2_ap(c, kt):
            j, cc = c // 2, c % 2
            off = (j * 4 + cc * 2 + kt) * 2
            return w2s[:, off:off + 2]

        state = {"pe_prev": None, "pe_cnt": 0, "act_cnt": 0, "dve_cnt": 0,
                 "first_evict": {"act": True, "dve": True}}
        add_op = mybir.AluOpType.add
        max_op = mybir.AluOpType.max

        def act_touch(src_ap):
            j = state["act_cnt"]
            state["act_cnt"] += 1
            return nc.scalar.activation(act_scr[0:1, j:j + 1], src_ap, copy_f)

        def dve_touch(src_ap):
            j = state["dve_cnt"]
            state["dve_cnt"] += 1
            return nc.vector.tensor_copy(dve_scr[0:1, j:j + 1], src_ap)

        def pe_pin(inst):
            # no-sync scheduler edge chaining PE program order
            if state["pe_prev"] is not None:
                add_dep_helper(inst.ins, state["pe_prev"].ins, False, "pe order")
            state["pe_prev"] = inst
            return inst

        def pe_touch(col_ap):
            # Standalone LDWEIGHTS: a PE instruction that reads the AP (so it
            # absorbs one sem wait) without touching PSUM.  The next real
            # matmul self-loads its own weights, so the clobber is harmless.
            state["pe_cnt"] += 1
            if col_ap.dtype in (mybir.dt.float32, mybir.dt.float32r):
                col_ap = col_ap.bitcast(mybir.dt.bfloat16)
            t = nc.tensor.ldweights(col_ap)
            return pe_pin(t)

        z_lhs = zeros[:, 0:128]
        z_rhs = zeros[:, 128:128 + BC]
        if wdt == mybir.dt.float32r:
            z_lhs = z_lhs.bitcast(f32)
            z_rhs = z_rhs.bitcast(f32)

        # PE warm-up while the first DMAs land.
        pswarm = ps0p.tile([128, BC], f32, name="ps0")
        for _ in range(NWARM):
            pe_pin(nc.tensor.matmul(
                pswarm[:, :], lhsT=z_lhs, rhs=z_rhs, start=True, stop=True))

        act_bias_touch = act_touch(bias[0:1, 0:1])
        dve_bias_touch = dve_touch(bias[0:1, 0:1])

        def evict(engine, dst_ap, src_ps, bias_ap, with_relu=True,
                  touch_first=False):
            # A slot-reusing evict would carry two waits (own-sem slot release
            # + PE psum); the touch absorbs the PE wait first.
            t = None
            if touch_first:
                t = act_touch(src_ps[0:1, 0:1]) if engine == "act" \
                    else dve_touch(src_ps[0:1, 0:1])
            if engine == "act":
                if with_relu:
                    e = nc.scalar.activation(dst_ap, src_ps, relu, bias=bias_ap)
                else:
                    e = nc.scalar.add(dst_ap, src_ps, bias_ap)
            else:
                e = nc.vector.tensor_scalar(
                    dst_ap, src_ps, bias_ap, 0.0, add_op, max_op)
            if engine == "dve":
                state["last_dve"] = e
            if t is not None:
                add_dep_helper(e.ins, t.ins, False, "touch before evict")
            if state["first_evict"][engine]:
                state["first_evict"][engine] = False
                bt = act_bias_touch if engine == "act" else dve_bias_touch
                add_dep_helper((t or e).ins, bt.ins, False, "bias touch first")
            return e

        last_act = None
        pending = None  # deferred (bi, ps2, h1s) layer-2 batch

        def emit_l2(pbi, ps2, h1s):
            nonlocal last_act
            if pbi == 0:
                w2t = pe_touch(w2s[:, 0:1])
            if pbi >= 1:
                # Absorb the ps2 WAR (ACT out-evict of pbi-1) so the first
                # L2 matmul carries only its h1 (DVE) wait.
                pe_touch(outt[0:1, pbi - 1, 0:1])
            for c in range(C):
                j, cc = c // 2, c % 2
                for kt in range(KT2):
                    mm2 = pe_pin(nc.tensor.matmul(
                        ps2[32 * j:32 * j + 2, :],
                        lhsT=w2_ap(c, kt),
                        rhs=h1s[c][:, kt, :],
                        start=(cc == 0 and kt == 0),
                        stop=(cc == 1 and kt == KT2 - 1),
                        tile_position=(0, 32 * j),
                    ))
                    if pbi == 0 and c == 0 and kt == 0:
                        add_dep_helper(mm2.ins, w2t.ins, False, "w2t first")
            last_act = evict("act", outt[:, pbi, :], ps2, b2_ap,
                             with_relu=False)
            gpt = nc.gpsimd.tensor_copy(
                gp_scr[0:1, pbi:pbi + 1], outt[0:1, pbi, 0:1])
            od0 = nc.gpsimd.dma_start(
                out=outT2[0][:, pbi * BC:(pbi + 1) * BC],
                in_=outt[0:97:32, pbi, :],
            )
            add_dep_helper(od0.ins, gpt.ins, False, "gp touch before dma")
            od1 = nc.gpsimd.dma_start(
                out=outT2[1][:, pbi * BC:(pbi + 1) * BC],
                in_=outt[1:98:32, pbi, :],
            )
            add_dep_helper(od1.ins, gpt.ins, False, "gp touch before dma")
            funnel_deps.extend([od0, od1])

        for bi in range(NB):
            ps2 = ps2p.tile([128, BC], f32)
            if bi == 0:
                # zero the full bank once so the out-evict never reads
                # uninitialized PSUM in the unused partition rows
                pe_pin(nc.tensor.matmul(
                    ps2[:, :], lhsT=z_lhs, rhs=z_rhs, start=True, stop=True))
            xt = pe_touch(xs[:, bi, 0, 0:1])
            h1s = []
            h0s = {}
            # Software pipeline: phase ph runs L0 of c=ph and L1 of c=ph-1,
            # so the ACT evicts of h0[c] get a full extra phase (~1.7us)
            # before the L1 matmuls consume them.
            for ph in range(C + 1):
                if ph == 1 and pending is not None:
                    # Deferred layer-2 batch of the previous bi: running it
                    # after this bi's first L0 block hides the h1(c7) evict
                    # latency behind ~1.7us of layer-0 matmuls.
                    emit_l2(*pending)
                    pending = None
                if ph < C:
                    c = ph
                    if bi == 0:
                        wt = pe_touch(wts[c][:, 0:1])
                    h0 = h0p.tile([128, KT1, BC], wdt)
                    h0s[c] = h0
                    for mt in range(MT0):
                        ps0 = ps0p.tile([128, BC], f32)
                        for kt in range(KT0):
                            mmi = pe_pin(nc.tensor.matmul(
                                ps0,
                                lhsT=w0_ap(c, kt, mt),
                                rhs=xs_ap(kt, bi),
                                start=(kt == 0),
                                stop=(kt == KT0 - 1),
                            ))
                            if bi == 0 and mt == 0 and kt == 0:
                                add_dep_helper(mmi.ins, wt.ins, False, "wt first")
                            if c == 0 and mt == 0 and kt == 0:
                                add_dep_helper(mmi.ins, xt.ins, False, "xt first")
                        evict("act", h0[:, mt, :], ps0, b0_ap(c, mt),
                              touch_first=(mt == 0))
                if ph >= 1:
                    c = ph - 1
                    h0 = h0s.pop(c)
                    if bi == 0:
                        wt1 = pe_touch(wts[c][:, W0C:W0C + 1])
                    # Absorb the ACT (h0 write) wait on PE so the first L1
                    # matmul carries only its ps1 slot-release (DVE) wait.
                    h0t = pe_touch(h0[0:1, 0, 0:1])
                    h1 = h1p.tile([128, KT2, BC], wdt)
                    h1s.append(h1)
                    for mt in range(MT1):
                        ps1 = ps1p.tile([128, BC], f32)
                        for kt in range(KT1):
                            mm1 = pe_pin(nc.tensor.matmul(
                                ps1,
                                lhsT=w1_ap(c, kt, mt),
                                rhs=h0[:, kt, :],
                                start=(kt == 0),
                                stop=(kt == KT1 - 1),
                            ))
                            if bi == 0 and mt == 0 and kt == 0:
                                add_dep_helper(mm1.ins, wt1.ins, False, "w1t first")
                            if mt == 0 and kt == 0:
                                add_dep_helper(mm1.ins, h0t.ins, False, "h0t first")
                        evict("dve", h1[:, mt, :], ps1, b1_ap(c, mt),
                              touch_first=(mt == 0))
            pending = (bi, ps2, h1s)

        emit_l2(*pending)
        funnel_deps += [mset, last_act, state["last_dve"], state["pe_prev"]]
        for dep in funnel_deps:
            n = nc.sync.nop()
            add_dep_helper(n.ins, dep.ins, True, "drain funnel")
    return nc


def _np_wdt(mm: str):
    if mm == "bf16":
        import ml_dtypes

        return ml_dtypes.bfloat16
    return np.float32


def kernel(x, W0, b0, W1, b1, W2, b2, trace=False):
    mm = MM_MODE
    key = ("nc", mm)
    if key not in _CACHE:
        _CACHE[key] = _build(mm)
    nc = _CACHE[key]
    wnp = _np_wdt(mm)

    x = np.ascontiguousarray(np.asarray(x, dtype=np.float32))
    W0 = np.asarray(W0, dtype=np.float32)
    W1 = np.asarray(W1, dtype=np.float32)
    W2 = np.asarray(W2, dtype=np.float32)
    b0 = np.asarray(b0, dtype=np.float32)
    b1 = np.asarray(b1, dtype=np.float32)
    b2 = np.asarray(b2, dtype=np.float32)

    # Combined per-c weight block: [C, 128, KT0*H0 + KT1*H1] where
    # wcat[c, p, kt*H0 + h] = W0[c, h, kt*128+p] and
    # wcat[c, p, KT0*H0 + kt*H1 + o] = W1[c, o, kt*128+p].
    wcat = np.empty((C, 128, WCOLS), dtype=np.float32)
    w0v = wcat[:, :, :W0C].reshape(C, 128, KT0, H0)
    w0v[...] = W0.reshape(C, H0, KT0, 128).transpose(0, 3, 2, 1)
    w1v = wcat[:, :, W0C:].reshape(C, 128, KT1, H1)
    w1v[...] = W1.reshape(C, H1, KT1, 128).transpose(0, 3, 2, 1)
    wcat = np.ascontiguousarray(wcat).astype(wnp)

    # Layer-2 col-tiled lhsT tiles: for (c, kt) a [128, 2] tile at column
    # (j*4 + cc*2 + kt)*2, with the W2 slice in column m=cc, 0 in the other.
    w2part = np.zeros((128, W2_COLS), dtype=np.float32)
    for c in range(C):
        j, cc = c // 2, c % 2
        for kt in range(KT2):
            col = (j * 4 + cc * 2 + kt) * 2 + cc
            w2part[:, col] = W2[c, 0, kt * 128:(kt + 1) * 128]
    w2part = w2part.astype(wnp)

    biasd = np.zeros((128, BIAS_COLS), dtype=np.float32)
    biasd[:, B0_OFF:B0_OFF + C * MT0] = (
        b0.reshape(C, MT0, 128).transpose(2, 0, 1).reshape(128, C * MT0)
    )
    biasd[:, B1_OFF:B1_OFF + C * MT1] = (
        b1.reshape(C, MT1, 128).transpose(2, 0, 1).reshape(128, C * MT1)
    )
    for c in range(C):
        biasd[32 * (c // 2) + c % 2, B2_OFF] = b2[c]

    xTfull = np.ascontiguousarray(x.T)  # [D, B] fp32
    in_maps = []
    for s in range(NCORES):
        xsh = xTfull[:, s * BS:(s + 1) * BS]          # [D, BS]
        # xd[p, bi, kt, b] = x[s*BS + bi*BC + b, kt*128 + p]
        xdn = np.ascontiguousarray(
            xsh.reshape(KT0, 128, NB, BC).transpose(1, 2, 0, 3)
        ).astype(wnp)
        in_maps.append(
            {"xd": xdn, "w2d": w2part, "wcat": wcat, "biasd": biasd}
        )

    res = run_bass_kernel_spmd(
        nc, in_maps, core_ids=list(range(NCORES)), trace=trace
    )
    _CACHE["last_result"] = res

    out = np.empty((B, C), dtype=np.float32)
    for s in range(NCORES):
        o2 = res.results[s]["outT2"]  # [2, 4, BS]
        for c in range(C):
            out[s * BS:(s + 1) * BS, c] = o2[c % 2, c // 2]
    return out
